# revision 18
# baseline (speedup 1.0000x reference)
"""Causal multi-head self-attention with RoPE on 8 Trainium2 NeuronCores.

Full-input contract: kernel(**inputs) takes the complete tensors and returns
the complete [B, S, D] output. Internally shards (batch x head-group) across
8 cores: core c handles batch c//2 and heads (c%2)*8 .. (c%2)*8+8. Each core
computes its 8 heads' attention and a partial output projection; a pairwise
AllReduce (cores 2b, 2b+1) completes the projection sum.

All device layouts are transposed (feature-major) so no on-device transposes
are needed. RoPE even/odd pairs are separated via a host-side permutation of
the q/k weight rows, making RoPE pure full-width elementwise work.
"""

import numpy as np

import concourse.bass as bass
import concourse.mybir as mybir
import concourse.tile as tile
from concourse import bacc
from concourse.bass_utils import run_bass_kernel_spmd

F32 = mybir.dt.float32
F32R = mybir.dt.float32r
AF = mybir.ActivationFunctionType
ALU = mybir.AluOpType

P = 128          # partitions
SQ = 512         # moving-dim chunk (max for 4-byte dtypes)
DK = 64          # head dim
NH = 8           # heads per core
DLOC = NH * DK   # 512 local out-features for q/k/v
THETA = 10000.0

B, S, D, H = 4, 2048, 1024, 16
N_CORES = 8


def build_attention_program(DIN=D, DOUT=D, SEQ=S, all_reduce=True, groups=None, reps=1):
    """One SPMD Bass program. Per-core DRAM I/O (all float32):
      xt   [DIN, SEQ]    x[b].T
      wqt  [DIN, DLOC]   wq rows (perm: per-half E-block/O-block) transposed
      wkt  [DIN, DLOC]   likewise
      wvt  [DIN, DLOC]   wv rows (perm2: per-head [even|odd]) transposed
      wot  [DLOC, DOUT]  wo cols (perm2) transposed
      cos4 [P, SEQ]      cos table, 4x stacked [32, SEQ]
      sin4 [P, SEQ]
      outp [SEQ//SQ, DOUT, SQ]   out_partial^T, j-chunked
    """
    KC = DIN // P        # contraction chunks
    SJ = SEQ // SQ       # sq chunks
    STJ = SQ // P        # 128-s-tiles per sq chunk (4)
    ST = SEQ // P        # total s-tiles
    OC = DOUT // P       # out-proj dout chunks
    assert DIN % P == 0 and SEQ % SQ == 0

    nc = bacc.Bacc(
        "TRN2",
        target_bir_lowering=False,
        debug=False,
        num_devices=(len(groups) * len(groups[0]) if groups else 1),
    )
    xt = nc.declare_dram_parameter("xt", [DIN, SEQ], F32R, isOutput=False)
    wqt = nc.declare_dram_parameter("wqt", [DIN, DLOC], F32R, isOutput=False)
    wkt = nc.declare_dram_parameter("wkt", [DIN, DLOC], F32R, isOutput=False)
    wvt = nc.declare_dram_parameter("wvt", [DIN, DLOC], F32R, isOutput=False)
    wot = nc.declare_dram_parameter("wot", [DLOC, DOUT], F32R, isOutput=False)
    cos4 = nc.declare_dram_parameter("cos4", [P, SEQ], F32, isOutput=False)
    sin4 = nc.declare_dram_parameter("sin4", [P, SEQ], F32, isOutput=False)
    outp = nc.declare_dram_parameter("outp", [SJ, DOUT, SQ], F32, isOutput=True)

    from contextlib import ExitStack

    with tile.TileContext(nc) as tc, ExitStack() as ctx:
        ctx.enter_context(nc.allow_low_precision(reason="f32r carries full fp32 bytes"))
        consts = ctx.enter_context(tc.tile_pool(name="consts", bufs=1))
        tabs = ctx.enter_context(tc.tile_pool(name="tabs", bufs=1))
        wload = ctx.enter_context(tc.tile_pool(name="wload", bufs=1))
        xload = ctx.enter_context(tc.tile_pool(name="xload", bufs=2))
        qk_pool = ctx.enter_context(tc.tile_pool(name="qk", bufs=1))
        v_pool = ctx.enter_context(tc.tile_pool(name="vp", bufs=1))
        ot_pool = ctx.enter_context(tc.tile_pool(name="ot", bufs=1))
        tmp_pool = ctx.enter_context(tc.tile_pool(name="tmp", bufs=2))
        pt_pool = ctx.enter_context(tc.tile_pool(name="pt", bufs=6))
        den_pool = ctx.enter_context(tc.tile_pool(name="den", bufs=2))
        ob_pool = ctx.enter_context(tc.tile_pool(name="ob", bufs=2))
        dram_pool = ctx.enter_context(tc.tile_pool(name="dram", bufs=2, space="DRAM"))
        psA = ctx.enter_context(tc.tile_pool(name="psA", bufs=1, space="PSUM"))
        psS = ctx.enter_context(tc.tile_pool(name="psS", bufs=3, space="PSUM"))
        psV = ctx.enter_context(tc.tile_pool(name="psV", bufs=4, space="PSUM"))
        if True:
            for _rep in range(reps):
                ones_f32 = consts.tile([P, 1], F32, tag="one1")
                nc.vector.memset(ones_f32[:], 1.0)
                ones_sb = consts.tile([1, DK], F32R, tag="ones")
                nc.vector.tensor_copy(
                    ones_sb[:], ones_f32[0:1, 0:1].broadcast_to((1, DK))
                )

                # output-transposed o accumulator: 4 chunks of [P, SEQ]
                # rows: head h -> chunk h//2, base (h%2)*64, per-head order [E|O]
                ot_sb = [
                    ot_pool.tile([P, SEQ], F32R, tag=f"ot{i}", name=f"ot{i}")
                    for i in range(4)
                ]

                for half in range(2):  # heads 4*half .. 4*half+4
                    c0 = half * 256  # column base in wqt/wkt/wvt for this half

                    # --- load this half's weight slices: per-k-chunk tiles ---
                    wq_sb = wload.tile([P, KC, 256], F32R, tag="wq")
                    nc.sync.dma_start(
                        wq_sb[:], wqt[:, c0 : c0 + 256].rearrange("(k p) c -> p k c", p=P)
                    )
                    wk_sb = wload.tile([P, KC, 256], F32R, tag="wk")
                    nc.sync.dma_start(
                        wk_sb[:], wkt[:, c0 : c0 + 256].rearrange("(k p) c -> p k c", p=P)
                    )
                    wv_sb = wload.tile([P, KC, 256], F32R, tag="wv")
                    nc.sync.dma_start(
                        wv_sb[:], wvt[:, c0 : c0 + 256].rearrange("(k p) c -> p k c", p=P)
                    )

                    # rotated q/k in [dout, s] layout; E chunk = even dims of the
                    # half's 4 heads (4 heads x 32), O chunk = odd dims
                    qE = qk_pool.tile([P, SEQ], F32R, tag="qE")
                    qO = qk_pool.tile([P, SEQ], F32R, tag="qO")
                    kE = qk_pool.tile([P, SEQ], F32R, tag="kE")
                    kO = qk_pool.tile([P, SEQ], F32R, tag="kO")
                    # v natural [s, dv]: per s-tile, per head: 64 dims + ones col
                    v_sb = v_pool.tile([P, ST, NH // 2, DK + 1], F32R, tag="v")
                    nc.vector.tensor_copy(
                        v_sb[:, :, :, DK : DK + 1],
                        ones_f32[:, None, None, :].broadcast_to((P, ST, NH // 2, 1)),
                    )

                    # ---------------- Phase A: q/k/v projections ----------------
                    for j in range(SJ):
                        js = slice(j * SQ, (j + 1) * SQ)
                        xt_sb = xload.tile([P, KC, SQ], F32R, tag="xt")
                        nc.sync.dma_start(
                            xt_sb[:], xt[:, js].rearrange("(k p) s -> p k s", p=P)
                        )
                        cos_j = tabs.tile([P, SQ], F32, tag="cosj")
                        nc.sync.dma_start(cos_j[:], cos4[:, js])
                        sin_j = tabs.tile([P, SQ], F32, tag="sinj")
                        nc.sync.dma_start(sin_j[:], sin4[:, js])

                        # q/k: psum[dout 128, s 512] accumulated over KC chunks
                        qkps = {}
                        for tname, wsb in (("q", wq_sb), ("k", wk_sb)):
                            for eo in range(2):  # 0=E chunk, 1=O chunk
                                ps = psA.tile([P, SQ], F32, tag="mm")
                                cc = eo * P
                                for kk in range(KC):
                                    nc.tensor.matmul(
                                        ps[:],
                                        lhsT=(wsb[:, kk, cc : cc + P]),
                                        rhs=(xt_sb[:, kk, :]),
                                        start=(kk == 0),
                                        stop=(kk == KC - 1),
                                    )
                                qkps[(tname, eo)] = ps

                        # RoPE: yE = cos*E - sin*O ; yO = sin*E + cos*O
                        for tname, dE, dO in (("q", qE, qO), ("k", kE, kO)):
                            psE, psO = qkps[(tname, 0)], qkps[(tname, 1)]
                            t1 = tmp_pool.tile([P, SQ], F32, tag="t1")
                            nc.vector.tensor_tensor(t1[:], cos_j[:], psE[:], ALU.mult)
                            t2 = tmp_pool.tile([P, SQ], F32, tag="t2")
                            nc.vector.tensor_tensor(t2[:], sin_j[:], psO[:], ALU.mult)
                            nc.vector.tensor_tensor(dE[:, js], t1[:], t2[:], ALU.subtract)
                            t3 = tmp_pool.tile([P, SQ], F32, tag="t1")
                            nc.vector.tensor_tensor(t3[:], sin_j[:], psE[:], ALU.mult)
                            t4 = tmp_pool.tile([P, SQ], F32, tag="t2")
                            nc.vector.tensor_tensor(t4[:], cos_j[:], psO[:], ALU.mult)
                            nc.vector.tensor_tensor(dO[:, js], t3[:], t4[:], ALU.add)

                        # v: psum[s 128, dv 256] per s-tile
                        for st in range(STJ):
                            ps = psA.tile([P, 256], F32, tag="mm")
                            for kk in range(KC):
                                nc.tensor.matmul(
                                    ps[:],
                                    lhsT=(xt_sb[:, kk, st * P : (st + 1) * P]),
                                    rhs=(wv_sb[:, kk, :]),
                                    start=(kk == 0),
                                    stop=(kk == KC - 1),
                                )
                            nc.vector.tensor_copy(
                                v_sb[:, j * STJ + st, :, 0:DK],
                                ps.rearrange("p (h d) -> p h d", h=NH // 2),
                            )

                        # ---- attention for this j (QKV of j+1 overlaps it) ----
                        # sk-tile-outer / head-inner: the 4 heads' K=32 score
                        # matmuls are adjacent in PE order on distinct 32-row
                        # strips (tile_position), so they run concurrently.
                        ntile = (j + 1) * STJ  # causal: sk-tiles 0..ntile-1
                        opvs = [
                            psV.tile([DK + 1, SQ], F32, tag="pv", name=f"pv{h}")
                            for h in range(4)
                        ]
                        for t in range(ntile):
                            ts_ = slice(t * P, (t + 1) * P)
                            pts = []
                            for h in range(4):
                                bp = h * 32
                                ssc = psS.tile([P, SQ], F32, tag="sc")
                                nc.tensor.matmul(
                                    ssc[:],
                                    lhsT=(kE[bp : bp + 32, ts_]),
                                    rhs=(qE[bp : bp + 32, js]),
                                    start=True,
                                    stop=False,
                                    tile_position=(bp, 0),
                                )
                                nc.tensor.matmul(
                                    ssc[:],
                                    lhsT=(kO[bp : bp + 32, ts_]),
                                    rhs=(qO[bp : bp + 32, js]),
                                    start=False,
                                    stop=True,
                                    tile_position=(bp, 0),
                                )
                                pt = pt_pool.tile([P, SQ], F32R, tag="pt")
                                nc.scalar.activation(pt[:], ssc[:], AF.Exp, scale=0.125)
                                if t >= ntile - STJ:
                                    # diagonal tile: zero where sq < sk
                                    nc.gpsimd.affine_select(
                                        out=pt[:],
                                        in_=pt[:],
                                        compare_op=ALU.is_ge,
                                        fill=0.0,
                                        base=j * SQ - t * P,
                                        pattern=[[1, SQ]],
                                        channel_multiplier=-1,
                                    )
                                pts.append(pt)
                            for h in range(4):
                                nc.tensor.matmul(
                                    opvs[h][:],
                                    lhsT=(v_sb[:, t, h, :]),
                                    rhs=(pts[h][:]),
                                    start=(t == 0),
                                    stop=(t == ntile - 1),
                                )
                        for h in range(4):
                            hh = half * 4 + h
                            opv = opvs[h]
                            # normalize: rows 0..63 divided by row 64
                            # (partition-broadcast of 1/denom via K=1 PE matmul)
                            den = den_pool.tile([1, SQ], F32R, tag="den")
                            nc.vector.reciprocal(den[:], opv[DK : DK + 1, :])
                            psb = psS.tile([DK, SQ], F32, tag="sc")
                            nc.tensor.matmul(
                                psb[:], lhsT=(ones_sb[:]), rhs=(den[:]),
                                start=True, stop=True,
                            )
                            denb = den_pool.tile([DK, SQ], F32, tag="denb")
                            nc.vector.tensor_copy(denb[:], psb[:])
                            nc.vector.tensor_tensor(
                                ot_sb[hh // 2][(hh % 2) * DK : (hh % 2 + 1) * DK, js],
                                opv[0:DK, :],
                                denb[:],
                                ALU.mult,
                            )

                # ---------------- Phase C: output projection (+ AllReduce) ----------------
                wo_sb = consts.tile([P, 4, DOUT], F32R, tag="wo")
                nc.sync.dma_start(wo_sb[:], wot.rearrange("(k p) c -> p k c", p=P))
                for j in range(SJ):
                    js = slice(j * SQ, (j + 1) * SQ)
                    op_dram = dram_pool.tile([DOUT, SQ], F32, tag="opart")
                    for dc in range(OC):
                        ps = psA.tile([P, SQ], F32, tag="mm")
                        for ic in range(4):
                            nc.tensor.matmul(
                                ps[:],
                                lhsT=(wo_sb[:, ic, dc * P : (dc + 1) * P]),
                                rhs=(ot_sb[ic][:, js]),
                                start=(ic == 0),
                                stop=(ic == 3),
                            )
                        ob = ob_pool.tile([P, SQ], F32, tag="ob")
                        nc.vector.tensor_copy(ob[:], ps[:])
                        nc.sync.dma_start(op_dram[dc * P : (dc + 1) * P, :], ob[:])
                    if all_reduce:
                        ar_dram = dram_pool.tile([DOUT, SQ], F32, tag="arout")
                        nc.gpsimd.collective_compute(
                            "AllReduce",
                            ALU.add,
                            replica_groups=groups,
                            ins=[op_dram.opt()],
                            outs=[ar_dram.opt()],
                        )
                        nc.sync.dma_start(outp[j], ar_dram[:])
                    else:
                        nc.sync.dma_start(outp[j], op_dram[:])

    nc.finalize()
    return nc


def make_perms():
    """perm (q/k): per half, E-block then O-block across the half's 4 heads.
    perm2 (v/wo): per head, [even dims | odd dims].
    Both are local to a core's 512 rows (caller adds the head-group offset)."""
    perm = []
    for half in range(2):
        for par in range(2):  # 0=E, 1=O
            for h in range(4 * half, 4 * half + 4):
                for i in range(32):
                    perm.append(h * DK + 2 * i + par)
    perm2 = []
    for h in range(NH):
        for par in range(2):
            for i in range(32):
                perm2.append(h * DK + 2 * i + par)
    return np.array(perm), np.array(perm2)


def make_tables(token_positions, SEQ):
    pos = np.asarray(token_positions).astype(np.float32)
    inv_freq = (1.0 / (THETA ** (np.arange(0, DK, 2, dtype=np.float32) / DK))).astype(
        np.float32
    )
    freqs = pos[:, None] * inv_freq[None, :]  # [S, 32]
    cosT = np.cos(freqs).T.astype(np.float32)  # [32, S]
    sinT = np.sin(freqs).T.astype(np.float32)
    return (
        np.ascontiguousarray(np.tile(cosT, (4, 1))),
        np.ascontiguousarray(np.tile(sinT, (4, 1))),
    )


def shard_inputs(x, token_positions, wq, wk, wv, wo):
    """Build the 8 per-core input maps."""
    perm, perm2 = make_perms()
    cos4, sin4 = make_tables(token_positions, x.shape[1])
    in_maps = []
    for c in range(N_CORES):
        b, hg = c // 2, c % 2
        rows = hg * DLOC
        gperm = perm + rows
        gperm2 = perm2 + rows
        in_maps.append(
            {
                "xt": np.ascontiguousarray(x[b].T),
                "wqt": np.ascontiguousarray(wq[gperm, :].T),
                "wkt": np.ascontiguousarray(wk[gperm, :].T),
                "wvt": np.ascontiguousarray(wv[gperm2, :].T),
                "wot": np.ascontiguousarray(wo[:, gperm2].T),
                "cos4": cos4,
                "sin4": sin4,
            }
        )
    return in_maps


# ======================================================================
# v2: bf16 datapath, pair-packed PV + col-tiled denominator matmuls,
# 2-bank-wide exps, mask-multiply instead of per-tile affine_select,
# AllGather of attention outputs (split output projection) instead of
# AllReduce of output partials, out-proj software-pipelined by one j.
# ======================================================================

BF16 = mybir.dt.bfloat16


def build_attention_program_v2(SEQ=S, groups=None, reps=1, all_gather=True):
    """Per-core DRAM I/O (bf16 unless noted):
      xt   [D, SEQ]      x[b].T
      wqt  [D, DLOC]     wq rows (perm: per-half E-block/O-block) transposed
      wkt  [D, DLOC]
      wvt  [D, DLOC]     wv rows (perm2: per-head [even|odd]) transposed
      wot  [D, DLOC]     wo[dout half, perm2f cols].T  (full contraction dim)
      cos4 [P, SEQ]      cos table, 4x stacked [32, SEQ]
      sin4 [P, SEQ]
      outp [SEQ//SQ, DLOC, SQ]  per-core half of out^T, j-chunked
    """
    DIN = D
    KC = DIN // P       # 8 contraction chunks for QKV
    SJ = SEQ // SQ      # 4 seq chunks
    STJ = SQ // P       # 4 sk-tiles per chunk
    ST = SEQ // P       # 16 sk-tiles total
    n_dev = (len(groups) * len(groups[0])) if groups else 1

    nc = bacc.Bacc("TRN2", target_bir_lowering=False, debug=False, num_devices=n_dev)
    xt = nc.declare_dram_parameter("xt", [DIN, SEQ], BF16, isOutput=False)
    wqt = nc.declare_dram_parameter("wqt", [DIN, DLOC], BF16, isOutput=False)
    wkt = nc.declare_dram_parameter("wkt", [DIN, DLOC], BF16, isOutput=False)
    wvt = nc.declare_dram_parameter("wvt", [DIN, DLOC], BF16, isOutput=False)
    wot = nc.declare_dram_parameter("wot", [DIN, DLOC], BF16, isOutput=False)
    cos4 = nc.declare_dram_parameter("cos4", [P, SEQ], BF16, isOutput=False)
    sin4 = nc.declare_dram_parameter("sin4", [P, SEQ], BF16, isOutput=False)
    outp = nc.declare_dram_parameter("outp", [SJ, DLOC, SQ], BF16, isOutput=True)

    from contextlib import ExitStack

    with tile.TileContext(nc) as tc, ExitStack() as ctx:
        ctx.enter_context(nc.allow_low_precision(reason="bf16 datapath"))
        consts = ctx.enter_context(tc.tile_pool(name="consts", bufs=1))
        wload = ctx.enter_context(tc.tile_pool(name="wload", bufs=1))
        xload = ctx.enter_context(tc.tile_pool(name="xload", bufs=2))
        qk_pool = ctx.enter_context(tc.tile_pool(name="qk", bufs=1))
        v_pool = ctx.enter_context(tc.tile_pool(name="vp", bufs=1))
        rc_pool = ctx.enter_context(tc.tile_pool(name="rc", bufs=2))
        tmp_pool = ctx.enter_context(tc.tile_pool(name="tmp", bufs=2))
        pt_pool = ctx.enter_context(tc.tile_pool(name="pt", bufs=6))
        den_pool = ctx.enter_context(tc.tile_pool(name="den", bufs=2))
        ot_pool = ctx.enter_context(tc.tile_pool(name="ot", bufs=2))
        og_pool = ctx.enter_context(tc.tile_pool(name="og", bufs=2))
        ob_pool = ctx.enter_context(tc.tile_pool(name="ob", bufs=2))
        dram_pool = ctx.enter_context(tc.tile_pool(name="dram", bufs=2, space="DRAM"))
        psA = ctx.enter_context(tc.tile_pool(name="psA", bufs=1, space="PSUM"))
        psS = ctx.enter_context(tc.tile_pool(name="psS", bufs=2, space="PSUM"))
        psO = ctx.enter_context(tc.tile_pool(name="psO", bufs=2, space="PSUM"))
        psD = ctx.enter_context(tc.tile_pool(name="psD", bufs=1, space="PSUM"))

        for _rep in range(reps):
            ones_f32 = consts.tile([P, 1], F32, tag="one1")
            nc.vector.memset(ones_f32[:], 1.0)
            ones_bf = consts.tile([P, 1], BF16, tag="onebf")
            nc.vector.tensor_copy(ones_bf[:], ones_f32[:])
            zero_f32 = consts.tile([2 * 32, 1], F32, tag="zero1")
            nc.vector.memset(zero_f32[:], 0.0)
            # norm broadcast matrix: out rows 0-63 <- den row 0, 64-127 <- row 32
            z2 = consts.tile([2 * 32, P], F32, tag="z2")
            nc.vector.memset(z2[:], 0.0)
            nc.vector.memset(z2[0:1, 0:DK], 1.0)
            nc.vector.memset(z2[32:33, DK : 2 * DK], 1.0)
            ones2 = consts.tile([2 * 32, P], F32R, tag="ones2")
            nc.vector.tensor_copy(ones2[:], z2[:])
            # causal masks for the 4 diagonal sk-tile offsets r:
            # keep (=1) where col c >= p + 128 r
            mask4 = consts.tile([P, STJ, SQ], BF16, tag="mask4")
            nc.vector.tensor_copy(
                mask4[:], ones_f32[:, 0:1, None].broadcast_to((P, STJ, SQ))
            )
            for r in range(STJ):
                nc.gpsimd.affine_select(
                    out=mask4[:, r, :],
                    in_=mask4[:, r, :],
                    compare_op=ALU.is_ge,
                    fill=0.0,
                    base=-P * r,
                    pattern=[[1, SQ]],
                    channel_multiplier=-1,
                )

            # weights, resident all rep
            wq_sb = wload.tile([P, KC, DLOC], BF16, tag="wq")
            nc.sync.dma_start(wq_sb[:], wqt.rearrange("(k p) c -> p k c", p=P))
            wk_sb = wload.tile([P, KC, DLOC], BF16, tag="wk")
            nc.sync.dma_start(wk_sb[:], wkt.rearrange("(k p) c -> p k c", p=P))
            wv_sb = wload.tile([P, KC, DLOC], BF16, tag="wv")
            nc.sync.dma_start(wv_sb[:], wvt.rearrange("(k p) c -> p k c", p=P))
            wo_sb = wload.tile([P, KC, DLOC], BF16, tag="wo")
            nc.sync.dma_start(wo_sb[:], wot.rearrange("(k p) c -> p k c", p=P))
            cos_sb = consts.tile([P, SEQ], BF16, tag="cos")
            nc.sync.dma_start(cos_sb[:], cos4[:, :])
            sin_sb = consts.tile([P, SEQ], BF16, tag="sin")
            nc.sync.dma_start(sin_sb[:], sin4[:, :])

            # rotated q/k per half in [dout, s] layout, bf16
            qE = [
                qk_pool.tile([P, SEQ], BF16, tag=f"qE{h}", name=f"qE{h}")
                for h in range(2)
            ]
            qO = [
                qk_pool.tile([P, SEQ], BF16, tag=f"qO{h}", name=f"qO{h}")
                for h in range(2)
            ]
            kE = [
                qk_pool.tile([P, SEQ], BF16, tag=f"kE{h}", name=f"kE{h}")
                for h in range(2)
            ]
            kO = [
                qk_pool.tile([P, SEQ], BF16, tag=f"kO{h}", name=f"kO{h}")
                for h in range(2)
            ]
            # v natural [s, dv]: per sk-tile, 8 heads x 64 dims
            v_sb = v_pool.tile([P, ST, NH, DK], BF16, tag="v")

            # out-proj pipelined one j behind; remember (agout, j) to drain
            pending = []

            def do_oproj(agout_t, jj):
                og_sb = og_pool.tile([P, KC, SQ], BF16, tag="og")
                nc.sync.dma_start(
                    og_sb[:], agout_t.rearrange("(k p) s -> p k s", p=P)
                )
                for dc in range(DLOC // P):
                    ps = psA.tile([P, SQ], F32, tag="mm")
                    for ic in range(KC):
                        nc.tensor.matmul(
                            ps[:],
                            lhsT=(wo_sb[:, ic, dc * P : (dc + 1) * P]),
                            rhs=(og_sb[:, ic, :]),
                            start=(ic == 0),
                            stop=(ic == KC - 1),
                        )
                    ob = ob_pool.tile([P, SQ], BF16, tag="ob")
                    nc.vector.tensor_copy(ob[:], ps[:])
                    nc.sync.dma_start(outp[jj, dc * P : (dc + 1) * P, :], ob[:])

            xt_tiles = {}

            def emit_xload(j):
                js = slice(j * SQ, (j + 1) * SQ)
                xt_sb = xload.tile([P, KC, SQ], BF16, tag="xt", name=f"xt{j}")
                nc.sync.dma_start(
                    xt_sb[:], xt[:, js].rearrange("(k p) s -> p k s", p=P)
                )
                xt_tiles[j] = xt_sb

            def emit_qk(j, half):
                js = slice(j * SQ, (j + 1) * SQ)
                xt_sb = xt_tiles[j]
                c0 = half * 256
                for tname, wsb, dE, dO in (
                    ("q", wq_sb, qE[half], qO[half]),
                    ("k", wk_sb, kE[half], kO[half]),
                ):
                    cEO = []
                    for eo in range(2):
                        ps = psA.tile([P, SQ], F32, tag="mm")
                        cc = c0 + eo * P
                        for kk in range(KC):
                            nc.tensor.matmul(
                                ps[:],
                                lhsT=(wsb[:, kk, cc : cc + P]),
                                rhs=(xt_sb[:, kk, :]),
                                start=(kk == 0),
                                stop=(kk == KC - 1),
                            )
                        cx = rc_pool.tile([P, SQ], BF16, tag="cx")
                        nc.vector.tensor_copy(cx[:], ps[:])
                        cEO.append(cx)
                    cE, cO = cEO
                    cos_j = cos_sb[:, js]
                    sin_j = sin_sb[:, js]
                    t1 = tmp_pool.tile([P, SQ], BF16, tag="t1")
                    nc.vector.tensor_tensor(t1[:], cos_j, cE[:], ALU.mult)
                    t2 = tmp_pool.tile([P, SQ], BF16, tag="t2")
                    nc.vector.tensor_tensor(t2[:], sin_j, cO[:], ALU.mult)
                    nc.vector.tensor_tensor(dE[:, js], t1[:], t2[:], ALU.subtract)
                    t3 = tmp_pool.tile([P, SQ], BF16, tag="t1")
                    nc.vector.tensor_tensor(t3[:], sin_j, cE[:], ALU.mult)
                    t4 = tmp_pool.tile([P, SQ], BF16, tag="t2")
                    nc.vector.tensor_tensor(t4[:], cos_j, cO[:], ALU.mult)
                    nc.vector.tensor_tensor(dO[:, js], t3[:], t4[:], ALU.add)

            def emit_v(j):
                xt_sb = xt_tiles[j]
                for st in range(STJ):
                    ps = psA.tile([P, DLOC], F32, tag="mm")
                    for kk in range(KC):
                        nc.tensor.matmul(
                            ps[:],
                            lhsT=(xt_sb[:, kk, st * P : (st + 1) * P]),
                            rhs=(wv_sb[:, kk, :]),
                            start=(kk == 0),
                            stop=(kk == KC - 1),
                        )
                    nc.vector.tensor_copy(
                        v_sb[:, j * STJ + st, :, :],
                        ps.rearrange("p (h d) -> p h d", h=NH),
                    )

            def emit_attn_half(j, half, ot_j):
                js = slice(j * SQ, (j + 1) * SQ)
                ntile = (j + 1) * STJ
                if True:
                    opvs = [
                        psO.tile([P, SQ], F32, tag="pv", name=f"pv{half}{i}")
                        for i in range(2)
                    ]
                    den_ps = psD.tile([P, SQ], F32, tag="dn", name=f"dn{half}")

                    def emit_scores(t):
                        ts_ = slice(t * P, (t + 1) * P)
                        pts = []
                        for i in range(2):  # head pair
                            ssc = psS.tile([P, 2, SQ], F32, tag="sc")
                            for hh in range(2):
                                bp = (i * 2 + hh) * 32
                                nc.tensor.matmul(
                                    ssc[:, hh, :],
                                    lhsT=(kE[half][bp : bp + 32, ts_]),
                                    rhs=(qE[half][bp : bp + 32, js]),
                                    start=True,
                                    stop=False,
                                    tile_position=(bp, 0),
                                )
                                nc.tensor.matmul(
                                    ssc[:, hh, :],
                                    lhsT=(kO[half][bp : bp + 32, ts_]),
                                    rhs=(qO[half][bp : bp + 32, js]),
                                    start=False,
                                    stop=True,
                                    tile_position=(bp, 0),
                                )
                            pt = pt_pool.tile([P, 2, SQ], BF16, tag="pt")
                            nc.scalar.activation(pt[:], ssc[:], AF.Exp, scale=0.125)
                            if t >= ntile - STJ:
                                r = t - STJ * j
                                ptm = pt_pool.tile([P, 2, SQ], BF16, tag="pt")
                                nc.gpsimd.tensor_tensor(
                                    ptm[:],
                                    pt[:],
                                    mask4[:, r : r + 1, :].broadcast_to((P, 2, SQ)),
                                    ALU.mult,
                                )
                                pt = ptm
                            pts.append(pt)
                        return pts

                    def emit_pv(t, pts):
                        for i in range(2):
                            for hh in range(2):
                                hloc = i * 2 + hh
                                nc.tensor.matmul(
                                    opvs[i][hh * DK : (hh + 1) * DK, :],
                                    lhsT=(v_sb[:, t, half * 4 + hloc, :]),
                                    rhs=(pts[i][:, hh, :]),
                                    start=(t == 0),
                                    stop=(t == ntile - 1),
                                    tile_position=(0, hh * DK),
                                )
                                nc.tensor.matmul(
                                    den_ps[hloc * 32 : hloc * 32 + 1, :],
                                    lhsT=(ones_bf[:]),
                                    rhs=(pts[i][:, hh, :]),
                                    start=(t == 0),
                                    stop=(t == ntile - 1),
                                    tile_position=(0, hloc * 32),
                                )

                    # software pipeline: scores(t+1) issue before PV(t) so the
                    # PE never queue-blocks on exp(t)
                    prev = emit_scores(0)
                    for t in range(1, ntile):
                        cur = emit_scores(t)
                        emit_pv(t - 1, prev)
                        prev = cur
                    emit_pv(ntile - 1, prev)
                    # normalize: ot rows of pair i <- opv / den
                    for i in range(2):
                        den_sb = den_pool.tile([2 * 32, SQ], F32R, tag="dsb")
                        nc.vector.tensor_copy(
                            den_sb[:], zero_f32[:, 0:1].broadcast_to((2 * 32, SQ))
                        )
                        for hh in range(2):
                            hloc = i * 2 + hh
                            nc.vector.reciprocal(
                                den_sb[hh * 32 : hh * 32 + 1, :],
                                den_ps[hloc * 32 : hloc * 32 + 1, :],
                            )
                        psb = psS.tile([P, SQ], F32, tag="sc")
                        nc.tensor.matmul(
                            psb[:], lhsT=(ones2[:]), rhs=(den_sb[:]),
                            start=True, stop=True,
                        )
                        denb = den_pool.tile([P, SQ], F32, tag="denb")
                        nc.vector.tensor_copy(denb[:], psb[:])
                        nc.vector.tensor_tensor(
                            ot_j[:, half * 2 + i, :], opvs[i][:], denb[:], ALU.mult
                        )

            def emit_exchange(j, ot_j):
                if all_gather:
                    agin = dram_pool.tile([DLOC, SQ], BF16, tag="agin")
                    nc.sync.dma_start(
                        agin.rearrange("(c p) s -> p c s", p=P), ot_j[:]
                    )
                    agout = dram_pool.tile([2 * DLOC, SQ], BF16, tag="agout")
                    nc.gpsimd.collective_compute(
                        "AllGather",
                        ALU.bypass,
                        replica_groups=groups,
                        ins=[agin.opt()],
                        outs=[agout.opt()],
                    )
                    pending.append((agout, j))
                    if len(pending) > 1:
                        do_oproj(*pending.pop(0))
                else:
                    # single-core testing: duplicate own half
                    agout = dram_pool.tile([2 * DLOC, SQ], BF16, tag="agout")
                    nc.sync.dma_start(
                        agout[0:DLOC].rearrange("(c p) s -> p c s", p=P), ot_j[:]
                    )
                    nc.sync.dma_start(
                        agout[DLOC : 2 * DLOC].rearrange("(c p) s -> p c s", p=P),
                        ot_j[:],
                    )
                    pending.append((agout, j))
                    if len(pending) > 1:
                        do_oproj(*pending.pop(0))

            # ---- pipelined schedule: QKV(j+1) interleaves attention(j) ----
            emit_xload(0)
            emit_qk(0, 0)
            emit_qk(0, 1)
            emit_v(0)
            for j in range(SJ):
                if j + 1 < SJ:
                    emit_xload(j + 1)
                ot_j = ot_pool.tile([P, 4, SQ], BF16, tag="otj", name=f"otj{j}")
                emit_attn_half(j, 0, ot_j)
                if j + 1 < SJ:
                    emit_qk(j + 1, 0)
                emit_attn_half(j, 1, ot_j)
                if j + 1 < SJ:
                    emit_qk(j + 1, 1)
                    emit_v(j + 1)
                emit_exchange(j, ot_j)
                xt_tiles.pop(j, None)

            while pending:
                do_oproj(*pending.pop(0))

    nc.finalize()
    return nc


def make_perm2f():
    return np.array(
        [gh * DK + 2 * i + par for gh in range(H) for par in range(2) for i in range(32)]
    )


def shard_inputs_v2(x, token_positions, wq, wk, wv, wo):
    import ml_dtypes

    bf = ml_dtypes.bfloat16
    perm, perm2 = make_perms()
    perm2f = make_perm2f()
    cos4, sin4 = make_tables(token_positions, x.shape[1])
    in_maps = []
    for c in range(N_CORES):
        b, hg = c // 2, c % 2
        rows = hg * DLOC
        gperm = perm + rows
        gperm2 = perm2 + rows
        douts = np.arange(hg * DLOC, (hg + 1) * DLOC)
        in_maps.append(
            {
                "xt": np.ascontiguousarray(x[b].T).astype(bf),
                "wqt": np.ascontiguousarray(wq[gperm, :].T).astype(bf),
                "wkt": np.ascontiguousarray(wk[gperm, :].T).astype(bf),
                "wvt": np.ascontiguousarray(wv[gperm2, :].T).astype(bf),
                "wot": np.ascontiguousarray(wo[np.ix_(douts, perm2f)].T).astype(bf),
                "cos4": cos4.astype(bf),
                "sin4": sin4.astype(bf),
            }
        )
    return in_maps


def unshard_output_v2(res_list):
    out = np.empty((B, S, D), dtype=np.float32)
    for b in range(B):
        outT = np.empty((D, S), dtype=np.float32)
        for hg in range(2):
            chunks = res_list[2 * b + hg]["outp"]  # [SJ, DLOC, SQ] bf16
            for j in range(S // SQ):
                outT[hg * DLOC : (hg + 1) * DLOC, j * SQ : (j + 1) * SQ] = np.asarray(
                    chunks[j], dtype=np.float32
                )
        out[b] = outT.T
    return out


_NC_CACHE = {}


def kernel(x, token_positions, wq, wk, wv, wo, trace=False):
    x = np.asarray(x, dtype=np.float32)
    wq = np.asarray(wq, dtype=np.float32)
    wk = np.asarray(wk, dtype=np.float32)
    wv = np.asarray(wv, dtype=np.float32)
    wo = np.asarray(wo, dtype=np.float32)

    key = "v2"
    if key not in _NC_CACHE:
        _NC_CACHE[key] = build_attention_program_v2(
            SEQ=S,
            groups=[[0, 1], [2, 3], [4, 5], [6, 7]],
        )
    nc = _NC_CACHE[key]

    in_maps = shard_inputs_v2(x, token_positions, wq, wk, wv, wo)
    res = run_bass_kernel_spmd(nc, in_maps, list(range(N_CORES)), trace=trace)
    out = unshard_output_v2(res.results)
    if trace:
        return out, res
    return out



# revision 19
# speedup vs baseline: 1.2367x; 1.2367x over previous
"""Causal multi-head self-attention with RoPE on 8 Trainium2 NeuronCores.

Full-input contract: kernel(**inputs) takes the complete tensors and returns
the complete [B, S, D] output. Internally shards (batch x head-group) across
8 cores: core c handles batch c//2 and heads (c%2)*8 .. (c%2)*8+8. Each core
computes its 8 heads' attention and a partial output projection; a pairwise
AllReduce (cores 2b, 2b+1) completes the projection sum.

All device layouts are transposed (feature-major) so no on-device transposes
are needed. RoPE even/odd pairs are separated via a host-side permutation of
the q/k weight rows, making RoPE pure full-width elementwise work.
"""

import numpy as np

import concourse.bass as bass
import concourse.mybir as mybir
import concourse.tile as tile
from concourse import bacc
from concourse.bass_utils import run_bass_kernel_spmd

F32 = mybir.dt.float32
F32R = mybir.dt.float32r
AF = mybir.ActivationFunctionType
ALU = mybir.AluOpType

P = 128          # partitions
SQ = 512         # moving-dim chunk (max for 4-byte dtypes)
DK = 64          # head dim
NH = 8           # heads per core
DLOC = NH * DK   # 512 local out-features for q/k/v
THETA = 10000.0

B, S, D, H = 4, 2048, 1024, 16
N_CORES = 8


def build_attention_program(DIN=D, DOUT=D, SEQ=S, all_reduce=True, groups=None, reps=1):
    """One SPMD Bass program. Per-core DRAM I/O (all float32):
      xt   [DIN, SEQ]    x[b].T
      wqt  [DIN, DLOC]   wq rows (perm: per-half E-block/O-block) transposed
      wkt  [DIN, DLOC]   likewise
      wvt  [DIN, DLOC]   wv rows (perm2: per-head [even|odd]) transposed
      wot  [DLOC, DOUT]  wo cols (perm2) transposed
      cos4 [P, SEQ]      cos table, 4x stacked [32, SEQ]
      sin4 [P, SEQ]
      outp [SEQ//SQ, DOUT, SQ]   out_partial^T, j-chunked
    """
    KC = DIN // P        # contraction chunks
    SJ = SEQ // SQ       # sq chunks
    STJ = SQ // P        # 128-s-tiles per sq chunk (4)
    ST = SEQ // P        # total s-tiles
    OC = DOUT // P       # out-proj dout chunks
    assert DIN % P == 0 and SEQ % SQ == 0

    nc = bacc.Bacc(
        "TRN2",
        target_bir_lowering=False,
        debug=False,
        num_devices=(len(groups) * len(groups[0]) if groups else 1),
    )
    xt = nc.declare_dram_parameter("xt", [DIN, SEQ], F32R, isOutput=False)
    wqt = nc.declare_dram_parameter("wqt", [DIN, DLOC], F32R, isOutput=False)
    wkt = nc.declare_dram_parameter("wkt", [DIN, DLOC], F32R, isOutput=False)
    wvt = nc.declare_dram_parameter("wvt", [DIN, DLOC], F32R, isOutput=False)
    wot = nc.declare_dram_parameter("wot", [DLOC, DOUT], F32R, isOutput=False)
    cos4 = nc.declare_dram_parameter("cos4", [P, SEQ], F32, isOutput=False)
    sin4 = nc.declare_dram_parameter("sin4", [P, SEQ], F32, isOutput=False)
    outp = nc.declare_dram_parameter("outp", [SJ, DOUT, SQ], F32, isOutput=True)

    from contextlib import ExitStack

    with tile.TileContext(nc) as tc, ExitStack() as ctx:
        ctx.enter_context(nc.allow_low_precision(reason="f32r carries full fp32 bytes"))
        consts = ctx.enter_context(tc.tile_pool(name="consts", bufs=1))
        tabs = ctx.enter_context(tc.tile_pool(name="tabs", bufs=1))
        wload = ctx.enter_context(tc.tile_pool(name="wload", bufs=1))
        xload = ctx.enter_context(tc.tile_pool(name="xload", bufs=2))
        qk_pool = ctx.enter_context(tc.tile_pool(name="qk", bufs=1))
        v_pool = ctx.enter_context(tc.tile_pool(name="vp", bufs=1))
        ot_pool = ctx.enter_context(tc.tile_pool(name="ot", bufs=1))
        tmp_pool = ctx.enter_context(tc.tile_pool(name="tmp", bufs=2))
        pt_pool = ctx.enter_context(tc.tile_pool(name="pt", bufs=6))
        den_pool = ctx.enter_context(tc.tile_pool(name="den", bufs=2))
        ob_pool = ctx.enter_context(tc.tile_pool(name="ob", bufs=2))
        dram_pool = ctx.enter_context(tc.tile_pool(name="dram", bufs=2, space="DRAM"))
        psA = ctx.enter_context(tc.tile_pool(name="psA", bufs=1, space="PSUM"))
        psS = ctx.enter_context(tc.tile_pool(name="psS", bufs=3, space="PSUM"))
        psV = ctx.enter_context(tc.tile_pool(name="psV", bufs=4, space="PSUM"))
        if True:
            for _rep in range(reps):
                ones_f32 = consts.tile([P, 1], F32, tag="one1")
                nc.vector.memset(ones_f32[:], 1.0)
                ones_sb = consts.tile([1, DK], F32R, tag="ones")
                nc.vector.tensor_copy(
                    ones_sb[:], ones_f32[0:1, 0:1].broadcast_to((1, DK))
                )

                # output-transposed o accumulator: 4 chunks of [P, SEQ]
                # rows: head h -> chunk h//2, base (h%2)*64, per-head order [E|O]
                ot_sb = [
                    ot_pool.tile([P, SEQ], F32R, tag=f"ot{i}", name=f"ot{i}")
                    for i in range(4)
                ]

                for half in range(2):  # heads 4*half .. 4*half+4
                    c0 = half * 256  # column base in wqt/wkt/wvt for this half

                    # --- load this half's weight slices: per-k-chunk tiles ---
                    wq_sb = wload.tile([P, KC, 256], F32R, tag="wq")
                    nc.sync.dma_start(
                        wq_sb[:], wqt[:, c0 : c0 + 256].rearrange("(k p) c -> p k c", p=P)
                    )
                    wk_sb = wload.tile([P, KC, 256], F32R, tag="wk")
                    nc.sync.dma_start(
                        wk_sb[:], wkt[:, c0 : c0 + 256].rearrange("(k p) c -> p k c", p=P)
                    )
                    wv_sb = wload.tile([P, KC, 256], F32R, tag="wv")
                    nc.sync.dma_start(
                        wv_sb[:], wvt[:, c0 : c0 + 256].rearrange("(k p) c -> p k c", p=P)
                    )

                    # rotated q/k in [dout, s] layout; E chunk = even dims of the
                    # half's 4 heads (4 heads x 32), O chunk = odd dims
                    qE = qk_pool.tile([P, SEQ], F32R, tag="qE")
                    qO = qk_pool.tile([P, SEQ], F32R, tag="qO")
                    kE = qk_pool.tile([P, SEQ], F32R, tag="kE")
                    kO = qk_pool.tile([P, SEQ], F32R, tag="kO")
                    # v natural [s, dv]: per s-tile, per head: 64 dims + ones col
                    v_sb = v_pool.tile([P, ST, NH // 2, DK + 1], F32R, tag="v")
                    nc.vector.tensor_copy(
                        v_sb[:, :, :, DK : DK + 1],
                        ones_f32[:, None, None, :].broadcast_to((P, ST, NH // 2, 1)),
                    )

                    # ---------------- Phase A: q/k/v projections ----------------
                    for j in range(SJ):
                        js = slice(j * SQ, (j + 1) * SQ)
                        xt_sb = xload.tile([P, KC, SQ], F32R, tag="xt")
                        nc.sync.dma_start(
                            xt_sb[:], xt[:, js].rearrange("(k p) s -> p k s", p=P)
                        )
                        cos_j = tabs.tile([P, SQ], F32, tag="cosj")
                        nc.sync.dma_start(cos_j[:], cos4[:, js])
                        sin_j = tabs.tile([P, SQ], F32, tag="sinj")
                        nc.sync.dma_start(sin_j[:], sin4[:, js])

                        # q/k: psum[dout 128, s 512] accumulated over KC chunks
                        qkps = {}
                        for tname, wsb in (("q", wq_sb), ("k", wk_sb)):
                            for eo in range(2):  # 0=E chunk, 1=O chunk
                                ps = psA.tile([P, SQ], F32, tag="mm")
                                cc = eo * P
                                for kk in range(KC):
                                    nc.tensor.matmul(
                                        ps[:],
                                        lhsT=(wsb[:, kk, cc : cc + P]),
                                        rhs=(xt_sb[:, kk, :]),
                                        start=(kk == 0),
                                        stop=(kk == KC - 1),
                                    )
                                qkps[(tname, eo)] = ps

                        # RoPE: yE = cos*E - sin*O ; yO = sin*E + cos*O
                        for tname, dE, dO in (("q", qE, qO), ("k", kE, kO)):
                            psE, psO = qkps[(tname, 0)], qkps[(tname, 1)]
                            t1 = tmp_pool.tile([P, SQ], F32, tag="t1")
                            nc.vector.tensor_tensor(t1[:], cos_j[:], psE[:], ALU.mult)
                            t2 = tmp_pool.tile([P, SQ], F32, tag="t2")
                            nc.vector.tensor_tensor(t2[:], sin_j[:], psO[:], ALU.mult)
                            nc.vector.tensor_tensor(dE[:, js], t1[:], t2[:], ALU.subtract)
                            t3 = tmp_pool.tile([P, SQ], F32, tag="t1")
                            nc.vector.tensor_tensor(t3[:], sin_j[:], psE[:], ALU.mult)
                            t4 = tmp_pool.tile([P, SQ], F32, tag="t2")
                            nc.vector.tensor_tensor(t4[:], cos_j[:], psO[:], ALU.mult)
                            nc.vector.tensor_tensor(dO[:, js], t3[:], t4[:], ALU.add)

                        # v: psum[s 128, dv 256] per s-tile
                        for st in range(STJ):
                            ps = psA.tile([P, 256], F32, tag="mm")
                            for kk in range(KC):
                                nc.tensor.matmul(
                                    ps[:],
                                    lhsT=(xt_sb[:, kk, st * P : (st + 1) * P]),
                                    rhs=(wv_sb[:, kk, :]),
                                    start=(kk == 0),
                                    stop=(kk == KC - 1),
                                )
                            nc.vector.tensor_copy(
                                v_sb[:, j * STJ + st, :, 0:DK],
                                ps.rearrange("p (h d) -> p h d", h=NH // 2),
                            )

                        # ---- attention for this j (QKV of j+1 overlaps it) ----
                        # sk-tile-outer / head-inner: the 4 heads' K=32 score
                        # matmuls are adjacent in PE order on distinct 32-row
                        # strips (tile_position), so they run concurrently.
                        ntile = (j + 1) * STJ  # causal: sk-tiles 0..ntile-1
                        opvs = [
                            psV.tile([DK + 1, SQ], F32, tag="pv", name=f"pv{h}")
                            for h in range(4)
                        ]
                        for t in range(ntile):
                            ts_ = slice(t * P, (t + 1) * P)
                            pts = []
                            for h in range(4):
                                bp = h * 32
                                ssc = psS.tile([P, SQ], F32, tag="sc")
                                nc.tensor.matmul(
                                    ssc[:],
                                    lhsT=(kE[bp : bp + 32, ts_]),
                                    rhs=(qE[bp : bp + 32, js]),
                                    start=True,
                                    stop=False,
                                    tile_position=(bp, 0),
                                )
                                nc.tensor.matmul(
                                    ssc[:],
                                    lhsT=(kO[bp : bp + 32, ts_]),
                                    rhs=(qO[bp : bp + 32, js]),
                                    start=False,
                                    stop=True,
                                    tile_position=(bp, 0),
                                )
                                pt = pt_pool.tile([P, SQ], F32R, tag="pt")
                                nc.scalar.activation(pt[:], ssc[:], AF.Exp, scale=0.125)
                                if t >= ntile - STJ:
                                    # diagonal tile: zero where sq < sk
                                    nc.gpsimd.affine_select(
                                        out=pt[:],
                                        in_=pt[:],
                                        compare_op=ALU.is_ge,
                                        fill=0.0,
                                        base=j * SQ - t * P,
                                        pattern=[[1, SQ]],
                                        channel_multiplier=-1,
                                    )
                                pts.append(pt)
                            for h in range(4):
                                nc.tensor.matmul(
                                    opvs[h][:],
                                    lhsT=(v_sb[:, t, h, :]),
                                    rhs=(pts[h][:]),
                                    start=(t == 0),
                                    stop=(t == ntile - 1),
                                )
                        for h in range(4):
                            hh = half * 4 + h
                            opv = opvs[h]
                            # normalize: rows 0..63 divided by row 64
                            # (partition-broadcast of 1/denom via K=1 PE matmul)
                            den = den_pool.tile([1, SQ], F32R, tag="den")
                            nc.vector.reciprocal(den[:], opv[DK : DK + 1, :])
                            psb = psS.tile([DK, SQ], F32, tag="sc")
                            nc.tensor.matmul(
                                psb[:], lhsT=(ones_sb[:]), rhs=(den[:]),
                                start=True, stop=True,
                            )
                            denb = den_pool.tile([DK, SQ], F32, tag="denb")
                            nc.vector.tensor_copy(denb[:], psb[:])
                            nc.vector.tensor_tensor(
                                ot_sb[hh // 2][(hh % 2) * DK : (hh % 2 + 1) * DK, js],
                                opv[0:DK, :],
                                denb[:],
                                ALU.mult,
                            )

                # ---------------- Phase C: output projection (+ AllReduce) ----------------
                wo_sb = consts.tile([P, 4, DOUT], F32R, tag="wo")
                nc.sync.dma_start(wo_sb[:], wot.rearrange("(k p) c -> p k c", p=P))
                for j in range(SJ):
                    js = slice(j * SQ, (j + 1) * SQ)
                    op_dram = dram_pool.tile([DOUT, SQ], F32, tag="opart")
                    for dc in range(OC):
                        ps = psA.tile([P, SQ], F32, tag="mm")
                        for ic in range(4):
                            nc.tensor.matmul(
                                ps[:],
                                lhsT=(wo_sb[:, ic, dc * P : (dc + 1) * P]),
                                rhs=(ot_sb[ic][:, js]),
                                start=(ic == 0),
                                stop=(ic == 3),
                            )
                        ob = ob_pool.tile([P, SQ], F32, tag="ob")
                        nc.vector.tensor_copy(ob[:], ps[:])
                        nc.sync.dma_start(op_dram[dc * P : (dc + 1) * P, :], ob[:])
                    if all_reduce:
                        ar_dram = dram_pool.tile([DOUT, SQ], F32, tag="arout")
                        nc.gpsimd.collective_compute(
                            "AllReduce",
                            ALU.add,
                            replica_groups=groups,
                            ins=[op_dram.opt()],
                            outs=[ar_dram.opt()],
                        )
                        nc.sync.dma_start(outp[j], ar_dram[:])
                    else:
                        nc.sync.dma_start(outp[j], op_dram[:])

    nc.finalize()
    return nc


def make_perms():
    """perm (q/k): per half, E-block then O-block across the half's 4 heads.
    perm2 (v/wo): per head, [even dims | odd dims].
    Both are local to a core's 512 rows (caller adds the head-group offset)."""
    perm = []
    for half in range(2):
        for par in range(2):  # 0=E, 1=O
            for h in range(4 * half, 4 * half + 4):
                for i in range(32):
                    perm.append(h * DK + 2 * i + par)
    perm2 = []
    for h in range(NH):
        for par in range(2):
            for i in range(32):
                perm2.append(h * DK + 2 * i + par)
    return np.array(perm), np.array(perm2)


def make_tables(token_positions, SEQ):
    pos = np.asarray(token_positions).astype(np.float32)
    inv_freq = (1.0 / (THETA ** (np.arange(0, DK, 2, dtype=np.float32) / DK))).astype(
        np.float32
    )
    freqs = pos[:, None] * inv_freq[None, :]  # [S, 32]
    cosT = np.cos(freqs).T.astype(np.float32)  # [32, S]
    sinT = np.sin(freqs).T.astype(np.float32)
    return (
        np.ascontiguousarray(np.tile(cosT, (4, 1))),
        np.ascontiguousarray(np.tile(sinT, (4, 1))),
    )


def shard_inputs(x, token_positions, wq, wk, wv, wo):
    """Build the 8 per-core input maps."""
    perm, perm2 = make_perms()
    cos4, sin4 = make_tables(token_positions, x.shape[1])
    in_maps = []
    for c in range(N_CORES):
        b, hg = c // 2, c % 2
        rows = hg * DLOC
        gperm = perm + rows
        gperm2 = perm2 + rows
        in_maps.append(
            {
                "xt": np.ascontiguousarray(x[b].T),
                "wqt": np.ascontiguousarray(wq[gperm, :].T),
                "wkt": np.ascontiguousarray(wk[gperm, :].T),
                "wvt": np.ascontiguousarray(wv[gperm2, :].T),
                "wot": np.ascontiguousarray(wo[:, gperm2].T),
                "cos4": cos4,
                "sin4": sin4,
            }
        )
    return in_maps


# ======================================================================
# v2: bf16 datapath, pair-packed PV + col-tiled denominator matmuls,
# 2-bank-wide exps, mask-multiply instead of per-tile affine_select,
# AllGather of attention outputs (split output projection) instead of
# AllReduce of output partials, out-proj software-pipelined by one j.
# ======================================================================

BF16 = mybir.dt.bfloat16


def build_attention_program_v2(SEQ=S, groups=None, reps=1, all_gather=True):
    """Per-core DRAM I/O (bf16 unless noted):
      xt   [D, SEQ]      x[b].T
      wqt  [D, DLOC]     wq rows (perm: per-half E-block/O-block) transposed
      wkt  [D, DLOC]
      wvt  [D, DLOC]     wv rows (perm2: per-head [even|odd]) transposed
      wot  [D, DLOC]     wo[dout half, perm2f cols].T  (full contraction dim)
      cos4 [P, SEQ]      cos table, 4x stacked [32, SEQ]
      sin4 [P, SEQ]
      outp [SEQ//SQ, DLOC, SQ]  per-core half of out^T, j-chunked
    """
    DIN = D
    KC = DIN // P       # 8 contraction chunks for QKV
    SJ = SEQ // SQ      # 4 seq chunks
    STJ = SQ // P       # 4 sk-tiles per chunk
    ST = SEQ // P       # 16 sk-tiles total
    n_dev = (len(groups) * len(groups[0])) if groups else 1

    nc = bacc.Bacc("TRN2", target_bir_lowering=False, debug=False, num_devices=n_dev)
    xt = nc.declare_dram_parameter("xt", [DIN, SEQ], BF16, isOutput=False)
    wqt = nc.declare_dram_parameter("wqt", [DIN, DLOC], BF16, isOutput=False)
    wkt = nc.declare_dram_parameter("wkt", [DIN, DLOC], BF16, isOutput=False)
    wvt = nc.declare_dram_parameter("wvt", [DIN, DLOC], BF16, isOutput=False)
    wot = nc.declare_dram_parameter("wot", [DIN, DLOC], BF16, isOutput=False)
    cos4 = nc.declare_dram_parameter("cos4", [P, SEQ], BF16, isOutput=False)
    sin4 = nc.declare_dram_parameter("sin4", [P, SEQ], BF16, isOutput=False)
    outp = nc.declare_dram_parameter("outp", [SJ, DLOC, SQ], BF16, isOutput=True)

    from contextlib import ExitStack

    with tile.TileContext(nc) as tc, ExitStack() as ctx:
        ctx.enter_context(nc.allow_low_precision(reason="bf16 datapath"))
        consts = ctx.enter_context(tc.tile_pool(name="consts", bufs=1))
        wload = ctx.enter_context(tc.tile_pool(name="wload", bufs=1))
        xload = ctx.enter_context(tc.tile_pool(name="xload", bufs=2))
        qk_pool = ctx.enter_context(tc.tile_pool(name="qk", bufs=1))
        v_pool = ctx.enter_context(tc.tile_pool(name="vp", bufs=1))
        rc_pool = ctx.enter_context(tc.tile_pool(name="rc", bufs=2))
        tmp_pool = ctx.enter_context(tc.tile_pool(name="tmp", bufs=2))
        pt_pool = ctx.enter_context(tc.tile_pool(name="pt", bufs=6))
        den_pool = ctx.enter_context(tc.tile_pool(name="den", bufs=2))
        ot_pool = ctx.enter_context(tc.tile_pool(name="ot", bufs=2))
        og_pool = ctx.enter_context(tc.tile_pool(name="og", bufs=2))
        ob_pool = ctx.enter_context(tc.tile_pool(name="ob", bufs=2))
        dram_pool = ctx.enter_context(tc.tile_pool(name="dram", bufs=2, space="DRAM"))
        psA = ctx.enter_context(tc.tile_pool(name="psA", bufs=1, space="PSUM"))
        psS = ctx.enter_context(tc.tile_pool(name="psS", bufs=2, space="PSUM"))
        psO = ctx.enter_context(tc.tile_pool(name="psO", bufs=2, space="PSUM"))
        psD = ctx.enter_context(tc.tile_pool(name="psD", bufs=1, space="PSUM"))

        for _rep in range(reps):
            ones_f32 = consts.tile([P, 1], F32, tag="one1")
            nc.vector.memset(ones_f32[:], 1.0)
            ones_bf = consts.tile([P, 1], BF16, tag="onebf")
            nc.vector.tensor_copy(ones_bf[:], ones_f32[:])
            zero_f32 = consts.tile([2 * 32, 1], F32, tag="zero1")
            nc.vector.memset(zero_f32[:], 0.0)
            # norm broadcast matrix: out rows 0-63 <- den row 0, 64-127 <- row 32
            z2 = consts.tile([2 * 32, P], F32, tag="z2")
            nc.vector.memset(z2[:], 0.0)
            nc.vector.memset(z2[0:1, 0:DK], 1.0)
            nc.vector.memset(z2[32:33, DK : 2 * DK], 1.0)
            ones2 = consts.tile([2 * 32, P], F32R, tag="ones2")
            nc.vector.tensor_copy(ones2[:], z2[:])
            # causal masks for the 4 diagonal sk-tile offsets r:
            # keep (=1) where col c >= p + 128 r
            mask4 = consts.tile([P, STJ, SQ], BF16, tag="mask4")
            nc.vector.tensor_copy(
                mask4[:], ones_f32[:, 0:1, None].broadcast_to((P, STJ, SQ))
            )
            for r in range(STJ):
                nc.gpsimd.affine_select(
                    out=mask4[:, r, :],
                    in_=mask4[:, r, :],
                    compare_op=ALU.is_ge,
                    fill=0.0,
                    base=-P * r,
                    pattern=[[1, SQ]],
                    channel_multiplier=-1,
                )

            # weights, resident all rep
            wq_sb = wload.tile([P, KC, DLOC], BF16, tag="wq")
            nc.sync.dma_start(wq_sb[:], wqt.rearrange("(k p) c -> p k c", p=P))
            wk_sb = wload.tile([P, KC, DLOC], BF16, tag="wk")
            nc.sync.dma_start(wk_sb[:], wkt.rearrange("(k p) c -> p k c", p=P))
            wv_sb = wload.tile([P, KC, DLOC], BF16, tag="wv")
            nc.sync.dma_start(wv_sb[:], wvt.rearrange("(k p) c -> p k c", p=P))
            wo_sb = wload.tile([P, KC, DLOC], BF16, tag="wo")
            nc.sync.dma_start(wo_sb[:], wot.rearrange("(k p) c -> p k c", p=P))
            cos_sb = consts.tile([P, SEQ], BF16, tag="cos")
            nc.sync.dma_start(cos_sb[:], cos4[:, :])
            sin_sb = consts.tile([P, SEQ], BF16, tag="sin")
            nc.sync.dma_start(sin_sb[:], sin4[:, :])

            # rotated q/k per half in [dout, s] layout, bf16
            qE = [
                qk_pool.tile([P, SEQ], BF16, tag=f"qE{h}", name=f"qE{h}")
                for h in range(2)
            ]
            qO = [
                qk_pool.tile([P, SEQ], BF16, tag=f"qO{h}", name=f"qO{h}")
                for h in range(2)
            ]
            kE = [
                qk_pool.tile([P, SEQ], BF16, tag=f"kE{h}", name=f"kE{h}")
                for h in range(2)
            ]
            kO = [
                qk_pool.tile([P, SEQ], BF16, tag=f"kO{h}", name=f"kO{h}")
                for h in range(2)
            ]
            # v natural [s, dv]: per sk-tile, 8 heads x 64 dims
            v_sb = v_pool.tile([P, ST, NH, DK], BF16, tag="v")

            # out-proj pipelined one j behind; remember (agout, j) to drain
            pending = []

            def do_oproj(agout_t, jj):
                og_sb = og_pool.tile([P, KC, SQ], BF16, tag="og")
                nc.sync.dma_start(
                    og_sb[:], agout_t.rearrange("(k p) s -> p k s", p=P)
                )
                for dc in range(DLOC // P):
                    ps = psA.tile([P, SQ], F32, tag="mm")
                    for ic in range(KC):
                        nc.tensor.matmul(
                            ps[:],
                            lhsT=(wo_sb[:, ic, dc * P : (dc + 1) * P]),
                            rhs=(og_sb[:, ic, :]),
                            start=(ic == 0),
                            stop=(ic == KC - 1),
                        )
                    ob = ob_pool.tile([P, SQ], BF16, tag="ob")
                    nc.vector.tensor_copy(ob[:], ps[:])
                    nc.sync.dma_start(outp[jj, dc * P : (dc + 1) * P, :], ob[:])

            xt_tiles = {}

            def emit_xload(j):
                js = slice(j * SQ, (j + 1) * SQ)
                xt_sb = xload.tile([P, KC, SQ], BF16, tag="xt", name=f"xt{j}")
                nc.sync.dma_start(
                    xt_sb[:], xt[:, js].rearrange("(k p) s -> p k s", p=P)
                )
                xt_tiles[j] = xt_sb

            def emit_qk(j, half):
                js = slice(j * SQ, (j + 1) * SQ)
                xt_sb = xt_tiles[j]
                c0 = half * 256
                for tname, wsb, dE, dO in (
                    ("q", wq_sb, qE[half], qO[half]),
                    ("k", wk_sb, kE[half], kO[half]),
                ):
                    cEO = []
                    for eo in range(2):
                        ps = psA.tile([P, SQ], F32, tag="mm")
                        cc = c0 + eo * P
                        for kk in range(KC):
                            nc.tensor.matmul(
                                ps[:],
                                lhsT=(wsb[:, kk, cc : cc + P]),
                                rhs=(xt_sb[:, kk, :]),
                                start=(kk == 0),
                                stop=(kk == KC - 1),
                            )
                        cx = rc_pool.tile([P, SQ], BF16, tag="cx")
                        nc.vector.tensor_copy(cx[:], ps[:])
                        cEO.append(cx)
                    cE, cO = cEO
                    cos_j = cos_sb[:, js]
                    sin_j = sin_sb[:, js]
                    t1 = tmp_pool.tile([P, SQ], BF16, tag="t1")
                    nc.vector.tensor_tensor(t1[:], cos_j, cE[:], ALU.mult)
                    t2 = tmp_pool.tile([P, SQ], BF16, tag="t2")
                    nc.vector.tensor_tensor(t2[:], sin_j, cO[:], ALU.mult)
                    nc.vector.tensor_tensor(dE[:, js], t1[:], t2[:], ALU.subtract)
                    t3 = tmp_pool.tile([P, SQ], BF16, tag="t1")
                    nc.vector.tensor_tensor(t3[:], sin_j, cE[:], ALU.mult)
                    t4 = tmp_pool.tile([P, SQ], BF16, tag="t2")
                    nc.vector.tensor_tensor(t4[:], cos_j, cO[:], ALU.mult)
                    nc.vector.tensor_tensor(dO[:, js], t3[:], t4[:], ALU.add)

            def emit_v(j):
                xt_sb = xt_tiles[j]
                for st in range(STJ):
                    ps = psA.tile([P, DLOC], F32, tag="mm")
                    for kk in range(KC):
                        nc.tensor.matmul(
                            ps[:],
                            lhsT=(xt_sb[:, kk, st * P : (st + 1) * P]),
                            rhs=(wv_sb[:, kk, :]),
                            start=(kk == 0),
                            stop=(kk == KC - 1),
                        )
                    nc.vector.tensor_copy(
                        v_sb[:, j * STJ + st, :, :],
                        ps.rearrange("p (h d) -> p h d", h=NH),
                    )

            def emit_attn_half(j, half, ot_j):
                js = slice(j * SQ, (j + 1) * SQ)
                ntile = (j + 1) * STJ
                if True:
                    opvs = [
                        psO.tile([P, SQ], F32, tag="pv", name=f"pv{half}{i}")
                        for i in range(2)
                    ]
                    den_ps = psD.tile([P, SQ], F32, tag="dn", name=f"dn{half}")

                    def emit_scores(t):
                        ts_ = slice(t * P, (t + 1) * P)
                        pts = []
                        for i in range(2):  # head pair
                            ssc = psS.tile([P, 2, SQ], F32, tag="sc")
                            for hh in range(2):
                                bp = (i * 2 + hh) * 32
                                nc.tensor.matmul(
                                    ssc[:, hh, :],
                                    lhsT=(kE[half][bp : bp + 32, ts_]),
                                    rhs=(qE[half][bp : bp + 32, js]),
                                    start=True,
                                    stop=False,
                                    tile_position=(bp, 0),
                                )
                                nc.tensor.matmul(
                                    ssc[:, hh, :],
                                    lhsT=(kO[half][bp : bp + 32, ts_]),
                                    rhs=(qO[half][bp : bp + 32, js]),
                                    start=False,
                                    stop=True,
                                    tile_position=(bp, 0),
                                )
                            pt = pt_pool.tile([P, 2, SQ], BF16, tag="pt")
                            nc.scalar.activation(pt[:], ssc[:], AF.Exp, scale=0.125)
                            if t >= ntile - STJ:
                                r = t - STJ * j
                                ptm = pt_pool.tile([P, 2, SQ], BF16, tag="pt")
                                nc.vector.tensor_tensor(
                                    ptm[:],
                                    pt[:],
                                    mask4[:, r : r + 1, :].broadcast_to((P, 2, SQ)),
                                    ALU.mult,
                                )
                                pt = ptm
                            pts.append(pt)
                        return pts

                    def emit_pv(t, pts):
                        for i in range(2):
                            for hh in range(2):
                                hloc = i * 2 + hh
                                nc.tensor.matmul(
                                    opvs[i][hh * DK : (hh + 1) * DK, :],
                                    lhsT=(v_sb[:, t, half * 4 + hloc, :]),
                                    rhs=(pts[i][:, hh, :]),
                                    start=(t == 0),
                                    stop=(t == ntile - 1),
                                    tile_position=(0, hh * DK),
                                )
                                nc.tensor.matmul(
                                    den_ps[hloc * 32 : hloc * 32 + 1, :],
                                    lhsT=(ones_bf[:]),
                                    rhs=(pts[i][:, hh, :]),
                                    start=(t == 0),
                                    stop=(t == ntile - 1),
                                    tile_position=(0, hloc * 32),
                                )

                    # software pipeline: scores(t+1) issue before PV(t) so the
                    # PE never queue-blocks on exp(t)
                    prev = emit_scores(0)
                    for t in range(1, ntile):
                        cur = emit_scores(t)
                        emit_pv(t - 1, prev)
                        prev = cur
                    emit_pv(ntile - 1, prev)
                    # normalize: ot rows of pair i <- opv / den
                    for i in range(2):
                        den_sb = den_pool.tile([2 * 32, SQ], F32R, tag="dsb")
                        nc.vector.tensor_copy(
                            den_sb[:], zero_f32[:, 0:1].broadcast_to((2 * 32, SQ))
                        )
                        for hh in range(2):
                            hloc = i * 2 + hh
                            nc.vector.reciprocal(
                                den_sb[hh * 32 : hh * 32 + 1, :],
                                den_ps[hloc * 32 : hloc * 32 + 1, :],
                            )
                        psb = psS.tile([P, SQ], F32, tag="sc")
                        nc.tensor.matmul(
                            psb[:], lhsT=(ones2[:]), rhs=(den_sb[:]),
                            start=True, stop=True,
                        )
                        denb = den_pool.tile([P, SQ], F32, tag="denb")
                        nc.vector.tensor_copy(denb[:], psb[:])
                        nc.vector.tensor_tensor(
                            ot_j[:, half * 2 + i, :], opvs[i][:], denb[:], ALU.mult
                        )

            def emit_exchange(j, ot_j):
                if all_gather:
                    agin = dram_pool.tile([DLOC, SQ], BF16, tag="agin")
                    nc.sync.dma_start(
                        agin.rearrange("(c p) s -> p c s", p=P), ot_j[:]
                    )
                    agout = dram_pool.tile([2 * DLOC, SQ], BF16, tag="agout")
                    nc.gpsimd.collective_compute(
                        "AllGather",
                        ALU.bypass,
                        replica_groups=groups,
                        ins=[agin.opt()],
                        outs=[agout.opt()],
                    )
                    pending.append((agout, j))
                    if len(pending) > 1:
                        do_oproj(*pending.pop(0))
                else:
                    # single-core testing: duplicate own half
                    agout = dram_pool.tile([2 * DLOC, SQ], BF16, tag="agout")
                    nc.sync.dma_start(
                        agout[0:DLOC].rearrange("(c p) s -> p c s", p=P), ot_j[:]
                    )
                    nc.sync.dma_start(
                        agout[DLOC : 2 * DLOC].rearrange("(c p) s -> p c s", p=P),
                        ot_j[:],
                    )
                    pending.append((agout, j))
                    if len(pending) > 1:
                        do_oproj(*pending.pop(0))

            # ---- pipelined schedule: QKV(j+1) interleaves attention(j) ----
            emit_xload(0)
            emit_qk(0, 0)
            emit_qk(0, 1)
            emit_v(0)
            for j in range(SJ):
                if j + 1 < SJ:
                    emit_xload(j + 1)
                ot_j = ot_pool.tile([P, 4, SQ], BF16, tag="otj", name=f"otj{j}")
                emit_attn_half(j, 0, ot_j)
                if j + 1 < SJ:
                    emit_qk(j + 1, 0)
                emit_attn_half(j, 1, ot_j)
                if j + 1 < SJ:
                    emit_qk(j + 1, 1)
                    emit_v(j + 1)
                emit_exchange(j, ot_j)
                xt_tiles.pop(j, None)

            while pending:
                do_oproj(*pending.pop(0))

    nc.finalize()
    return nc


def make_perm2f():
    return np.array(
        [gh * DK + 2 * i + par for gh in range(H) for par in range(2) for i in range(32)]
    )


def shard_inputs_v2(x, token_positions, wq, wk, wv, wo):
    import ml_dtypes

    bf = ml_dtypes.bfloat16
    perm, perm2 = make_perms()
    perm2f = make_perm2f()
    cos4, sin4 = make_tables(token_positions, x.shape[1])
    in_maps = []
    for c in range(N_CORES):
        b, hg = c // 2, c % 2
        rows = hg * DLOC
        gperm = perm + rows
        gperm2 = perm2 + rows
        douts = np.arange(hg * DLOC, (hg + 1) * DLOC)
        in_maps.append(
            {
                "xt": np.ascontiguousarray(x[b].T).astype(bf),
                "wqt": np.ascontiguousarray(wq[gperm, :].T).astype(bf),
                "wkt": np.ascontiguousarray(wk[gperm, :].T).astype(bf),
                "wvt": np.ascontiguousarray(wv[gperm2, :].T).astype(bf),
                "wot": np.ascontiguousarray(wo[np.ix_(douts, perm2f)].T).astype(bf),
                "cos4": cos4.astype(bf),
                "sin4": sin4.astype(bf),
            }
        )
    return in_maps


def unshard_output_v2(res_list):
    out = np.empty((B, S, D), dtype=np.float32)
    for b in range(B):
        outT = np.empty((D, S), dtype=np.float32)
        for hg in range(2):
            chunks = res_list[2 * b + hg]["outp"]  # [SJ, DLOC, SQ] bf16
            for j in range(S // SQ):
                outT[hg * DLOC : (hg + 1) * DLOC, j * SQ : (j + 1) * SQ] = np.asarray(
                    chunks[j], dtype=np.float32
                )
        out[b] = outT.T
    return out


_NC_CACHE = {}


def kernel(x, token_positions, wq, wk, wv, wo, trace=False):
    x = np.asarray(x, dtype=np.float32)
    wq = np.asarray(wq, dtype=np.float32)
    wk = np.asarray(wk, dtype=np.float32)
    wv = np.asarray(wv, dtype=np.float32)
    wo = np.asarray(wo, dtype=np.float32)

    key = "v2"
    if key not in _NC_CACHE:
        _NC_CACHE[key] = build_attention_program_v2(
            SEQ=S,
            groups=[[0, 1], [2, 3], [4, 5], [6, 7]],
        )
    nc = _NC_CACHE[key]

    in_maps = shard_inputs_v2(x, token_positions, wq, wk, wv, wo)
    res = run_bass_kernel_spmd(nc, in_maps, list(range(N_CORES)), trace=trace)
    out = unshard_output_v2(res.results)
    if trace:
        return out, res
    return out



# revision 20
# speedup vs baseline: 1.4452x; 1.1687x over previous
"""Causal multi-head self-attention with RoPE on 8 Trainium2 NeuronCores.

Full-input contract: kernel(**inputs) takes the complete tensors and returns
the complete [B, S, D] output. Internally shards (batch x head-group) across
8 cores: core c handles batch c//2 and heads (c%2)*8 .. (c%2)*8+8.

v2 (active, ~543us vs the 884us f32r baseline kept below as v1):
  - bf16 datapath end to end (f32 PSUM accumulate): FWL weight loads, 2-4x
    DVE element-wise, half the DMA/SBUF footprint; rel err ~8e-3 (< 2e-2).
  - Per j-chunk, each core's 8 heads run E/O-split score matmuls packed
    4-per-PE-array via tile_position row strips; PV matmuls packed 2-up via
    column strips (M=64 pairs sharing one PSUM tile); softmax denominators
    via M=1 col-tiled matmuls accumulated across sk-tiles.
  - Softmax exp on ACT over 2-PSUM-bank-wide [128, 1024] APs (two heads per
    instruction); causal diagonal handled by 4 precomputed bf16 masks
    (tensor_tensor mult on DVE) instead of per-tile affine_select.
  - In-order engine queues are fed software-pipelined: scores(t+1) issue
    before PV(t) so PE never blocks on exp(t); QKV(j+1) is interleaved
    between the two attention head-group halves of j; out-proj(j-1) runs
    behind the j-th AllGather.
  - Instead of AllReduce of output partials: per-j bf16 AllGather of the
    [512, 512] attention-output chunk between pair cores (2b, 2b+1), then
    each core projects the full 1024-dim contraction onto its half of the
    output features. Host concatenates.

All device layouts are transposed (feature-major) so no on-device transposes
are needed. RoPE even/odd pairs are separated via a host-side permutation of
the q/k weight rows, making RoPE pure full-width elementwise work.
"""

import numpy as np

import concourse.bass as bass
import concourse.mybir as mybir
import concourse.tile as tile
from concourse import bacc
from concourse.bass_utils import run_bass_kernel_spmd

F32 = mybir.dt.float32
F32R = mybir.dt.float32r
AF = mybir.ActivationFunctionType
ALU = mybir.AluOpType

P = 128          # partitions
SQ = 512         # moving-dim chunk (max for 4-byte dtypes)
DK = 64          # head dim
NH = 8           # heads per core
DLOC = NH * DK   # 512 local out-features for q/k/v
THETA = 10000.0

B, S, D, H = 4, 2048, 1024, 16
N_CORES = 8


def build_attention_program(DIN=D, DOUT=D, SEQ=S, all_reduce=True, groups=None, reps=1):
    """One SPMD Bass program. Per-core DRAM I/O (all float32):
      xt   [DIN, SEQ]    x[b].T
      wqt  [DIN, DLOC]   wq rows (perm: per-half E-block/O-block) transposed
      wkt  [DIN, DLOC]   likewise
      wvt  [DIN, DLOC]   wv rows (perm2: per-head [even|odd]) transposed
      wot  [DLOC, DOUT]  wo cols (perm2) transposed
      cos4 [P, SEQ]      cos table, 4x stacked [32, SEQ]
      sin4 [P, SEQ]
      outp [SEQ//SQ, DOUT, SQ]   out_partial^T, j-chunked
    """
    KC = DIN // P        # contraction chunks
    SJ = SEQ // SQ       # sq chunks
    STJ = SQ // P        # 128-s-tiles per sq chunk (4)
    ST = SEQ // P        # total s-tiles
    OC = DOUT // P       # out-proj dout chunks
    assert DIN % P == 0 and SEQ % SQ == 0

    nc = bacc.Bacc(
        "TRN2",
        target_bir_lowering=False,
        debug=False,
        num_devices=(len(groups) * len(groups[0]) if groups else 1),
    )
    xt = nc.declare_dram_parameter("xt", [DIN, SEQ], F32R, isOutput=False)
    wqt = nc.declare_dram_parameter("wqt", [DIN, DLOC], F32R, isOutput=False)
    wkt = nc.declare_dram_parameter("wkt", [DIN, DLOC], F32R, isOutput=False)
    wvt = nc.declare_dram_parameter("wvt", [DIN, DLOC], F32R, isOutput=False)
    wot = nc.declare_dram_parameter("wot", [DLOC, DOUT], F32R, isOutput=False)
    cos4 = nc.declare_dram_parameter("cos4", [P, SEQ], F32, isOutput=False)
    sin4 = nc.declare_dram_parameter("sin4", [P, SEQ], F32, isOutput=False)
    outp = nc.declare_dram_parameter("outp", [SJ, DOUT, SQ], F32, isOutput=True)

    from contextlib import ExitStack

    with tile.TileContext(nc) as tc, ExitStack() as ctx:
        ctx.enter_context(nc.allow_low_precision(reason="f32r carries full fp32 bytes"))
        consts = ctx.enter_context(tc.tile_pool(name="consts", bufs=1))
        tabs = ctx.enter_context(tc.tile_pool(name="tabs", bufs=1))
        wload = ctx.enter_context(tc.tile_pool(name="wload", bufs=1))
        xload = ctx.enter_context(tc.tile_pool(name="xload", bufs=2))
        qk_pool = ctx.enter_context(tc.tile_pool(name="qk", bufs=1))
        v_pool = ctx.enter_context(tc.tile_pool(name="vp", bufs=1))
        ot_pool = ctx.enter_context(tc.tile_pool(name="ot", bufs=1))
        tmp_pool = ctx.enter_context(tc.tile_pool(name="tmp", bufs=2))
        pt_pool = ctx.enter_context(tc.tile_pool(name="pt", bufs=6))
        den_pool = ctx.enter_context(tc.tile_pool(name="den", bufs=2))
        ob_pool = ctx.enter_context(tc.tile_pool(name="ob", bufs=2))
        dram_pool = ctx.enter_context(tc.tile_pool(name="dram", bufs=2, space="DRAM"))
        psA = ctx.enter_context(tc.tile_pool(name="psA", bufs=1, space="PSUM"))
        psS = ctx.enter_context(tc.tile_pool(name="psS", bufs=3, space="PSUM"))
        psV = ctx.enter_context(tc.tile_pool(name="psV", bufs=4, space="PSUM"))
        if True:
            for _rep in range(reps):
                ones_f32 = consts.tile([P, 1], F32, tag="one1")
                nc.vector.memset(ones_f32[:], 1.0)
                ones_sb = consts.tile([1, DK], F32R, tag="ones")
                nc.vector.tensor_copy(
                    ones_sb[:], ones_f32[0:1, 0:1].broadcast_to((1, DK))
                )

                # output-transposed o accumulator: 4 chunks of [P, SEQ]
                # rows: head h -> chunk h//2, base (h%2)*64, per-head order [E|O]
                ot_sb = [
                    ot_pool.tile([P, SEQ], F32R, tag=f"ot{i}", name=f"ot{i}")
                    for i in range(4)
                ]

                for half in range(2):  # heads 4*half .. 4*half+4
                    c0 = half * 256  # column base in wqt/wkt/wvt for this half

                    # --- load this half's weight slices: per-k-chunk tiles ---
                    wq_sb = wload.tile([P, KC, 256], F32R, tag="wq")
                    nc.sync.dma_start(
                        wq_sb[:], wqt[:, c0 : c0 + 256].rearrange("(k p) c -> p k c", p=P)
                    )
                    wk_sb = wload.tile([P, KC, 256], F32R, tag="wk")
                    nc.sync.dma_start(
                        wk_sb[:], wkt[:, c0 : c0 + 256].rearrange("(k p) c -> p k c", p=P)
                    )
                    wv_sb = wload.tile([P, KC, 256], F32R, tag="wv")
                    nc.sync.dma_start(
                        wv_sb[:], wvt[:, c0 : c0 + 256].rearrange("(k p) c -> p k c", p=P)
                    )

                    # rotated q/k in [dout, s] layout; E chunk = even dims of the
                    # half's 4 heads (4 heads x 32), O chunk = odd dims
                    qE = qk_pool.tile([P, SEQ], F32R, tag="qE")
                    qO = qk_pool.tile([P, SEQ], F32R, tag="qO")
                    kE = qk_pool.tile([P, SEQ], F32R, tag="kE")
                    kO = qk_pool.tile([P, SEQ], F32R, tag="kO")
                    # v natural [s, dv]: per s-tile, per head: 64 dims + ones col
                    v_sb = v_pool.tile([P, ST, NH // 2, DK + 1], F32R, tag="v")
                    nc.vector.tensor_copy(
                        v_sb[:, :, :, DK : DK + 1],
                        ones_f32[:, None, None, :].broadcast_to((P, ST, NH // 2, 1)),
                    )

                    # ---------------- Phase A: q/k/v projections ----------------
                    for j in range(SJ):
                        js = slice(j * SQ, (j + 1) * SQ)
                        xt_sb = xload.tile([P, KC, SQ], F32R, tag="xt")
                        nc.sync.dma_start(
                            xt_sb[:], xt[:, js].rearrange("(k p) s -> p k s", p=P)
                        )
                        cos_j = tabs.tile([P, SQ], F32, tag="cosj")
                        nc.sync.dma_start(cos_j[:], cos4[:, js])
                        sin_j = tabs.tile([P, SQ], F32, tag="sinj")
                        nc.sync.dma_start(sin_j[:], sin4[:, js])

                        # q/k: psum[dout 128, s 512] accumulated over KC chunks
                        qkps = {}
                        for tname, wsb in (("q", wq_sb), ("k", wk_sb)):
                            for eo in range(2):  # 0=E chunk, 1=O chunk
                                ps = psA.tile([P, SQ], F32, tag="mm")
                                cc = eo * P
                                for kk in range(KC):
                                    nc.tensor.matmul(
                                        ps[:],
                                        lhsT=(wsb[:, kk, cc : cc + P]),
                                        rhs=(xt_sb[:, kk, :]),
                                        start=(kk == 0),
                                        stop=(kk == KC - 1),
                                    )
                                qkps[(tname, eo)] = ps

                        # RoPE: yE = cos*E - sin*O ; yO = sin*E + cos*O
                        for tname, dE, dO in (("q", qE, qO), ("k", kE, kO)):
                            psE, psO = qkps[(tname, 0)], qkps[(tname, 1)]
                            t1 = tmp_pool.tile([P, SQ], F32, tag="t1")
                            nc.vector.tensor_tensor(t1[:], cos_j[:], psE[:], ALU.mult)
                            t2 = tmp_pool.tile([P, SQ], F32, tag="t2")
                            nc.vector.tensor_tensor(t2[:], sin_j[:], psO[:], ALU.mult)
                            nc.vector.tensor_tensor(dE[:, js], t1[:], t2[:], ALU.subtract)
                            t3 = tmp_pool.tile([P, SQ], F32, tag="t1")
                            nc.vector.tensor_tensor(t3[:], sin_j[:], psE[:], ALU.mult)
                            t4 = tmp_pool.tile([P, SQ], F32, tag="t2")
                            nc.vector.tensor_tensor(t4[:], cos_j[:], psO[:], ALU.mult)
                            nc.vector.tensor_tensor(dO[:, js], t3[:], t4[:], ALU.add)

                        # v: psum[s 128, dv 256] per s-tile
                        for st in range(STJ):
                            ps = psA.tile([P, 256], F32, tag="mm")
                            for kk in range(KC):
                                nc.tensor.matmul(
                                    ps[:],
                                    lhsT=(xt_sb[:, kk, st * P : (st + 1) * P]),
                                    rhs=(wv_sb[:, kk, :]),
                                    start=(kk == 0),
                                    stop=(kk == KC - 1),
                                )
                            nc.vector.tensor_copy(
                                v_sb[:, j * STJ + st, :, 0:DK],
                                ps.rearrange("p (h d) -> p h d", h=NH // 2),
                            )

                        # ---- attention for this j (QKV of j+1 overlaps it) ----
                        # sk-tile-outer / head-inner: the 4 heads' K=32 score
                        # matmuls are adjacent in PE order on distinct 32-row
                        # strips (tile_position), so they run concurrently.
                        ntile = (j + 1) * STJ  # causal: sk-tiles 0..ntile-1
                        opvs = [
                            psV.tile([DK + 1, SQ], F32, tag="pv", name=f"pv{h}")
                            for h in range(4)
                        ]
                        for t in range(ntile):
                            ts_ = slice(t * P, (t + 1) * P)
                            pts = []
                            for h in range(4):
                                bp = h * 32
                                ssc = psS.tile([P, SQ], F32, tag="sc")
                                nc.tensor.matmul(
                                    ssc[:],
                                    lhsT=(kE[bp : bp + 32, ts_]),
                                    rhs=(qE[bp : bp + 32, js]),
                                    start=True,
                                    stop=False,
                                    tile_position=(bp, 0),
                                )
                                nc.tensor.matmul(
                                    ssc[:],
                                    lhsT=(kO[bp : bp + 32, ts_]),
                                    rhs=(qO[bp : bp + 32, js]),
                                    start=False,
                                    stop=True,
                                    tile_position=(bp, 0),
                                )
                                pt = pt_pool.tile([P, SQ], F32R, tag="pt")
                                nc.scalar.activation(pt[:], ssc[:], AF.Exp, scale=0.125)
                                if t >= ntile - STJ:
                                    # diagonal tile: zero where sq < sk
                                    nc.gpsimd.affine_select(
                                        out=pt[:],
                                        in_=pt[:],
                                        compare_op=ALU.is_ge,
                                        fill=0.0,
                                        base=j * SQ - t * P,
                                        pattern=[[1, SQ]],
                                        channel_multiplier=-1,
                                    )
                                pts.append(pt)
                            for h in range(4):
                                nc.tensor.matmul(
                                    opvs[h][:],
                                    lhsT=(v_sb[:, t, h, :]),
                                    rhs=(pts[h][:]),
                                    start=(t == 0),
                                    stop=(t == ntile - 1),
                                )
                        for h in range(4):
                            hh = half * 4 + h
                            opv = opvs[h]
                            # normalize: rows 0..63 divided by row 64
                            # (partition-broadcast of 1/denom via K=1 PE matmul)
                            den = den_pool.tile([1, SQ], F32R, tag="den")
                            nc.vector.reciprocal(den[:], opv[DK : DK + 1, :])
                            psb = psS.tile([DK, SQ], F32, tag="sc")
                            nc.tensor.matmul(
                                psb[:], lhsT=(ones_sb[:]), rhs=(den[:]),
                                start=True, stop=True,
                            )
                            denb = den_pool.tile([DK, SQ], F32, tag="denb")
                            nc.vector.tensor_copy(denb[:], psb[:])
                            nc.vector.tensor_tensor(
                                ot_sb[hh // 2][(hh % 2) * DK : (hh % 2 + 1) * DK, js],
                                opv[0:DK, :],
                                denb[:],
                                ALU.mult,
                            )

                # ---------------- Phase C: output projection (+ AllReduce) ----------------
                wo_sb = consts.tile([P, 4, DOUT], F32R, tag="wo")
                nc.sync.dma_start(wo_sb[:], wot.rearrange("(k p) c -> p k c", p=P))
                for j in range(SJ):
                    js = slice(j * SQ, (j + 1) * SQ)
                    op_dram = dram_pool.tile([DOUT, SQ], F32, tag="opart")
                    for dc in range(OC):
                        ps = psA.tile([P, SQ], F32, tag="mm")
                        for ic in range(4):
                            nc.tensor.matmul(
                                ps[:],
                                lhsT=(wo_sb[:, ic, dc * P : (dc + 1) * P]),
                                rhs=(ot_sb[ic][:, js]),
                                start=(ic == 0),
                                stop=(ic == 3),
                            )
                        ob = ob_pool.tile([P, SQ], F32, tag="ob")
                        nc.vector.tensor_copy(ob[:], ps[:])
                        nc.sync.dma_start(op_dram[dc * P : (dc + 1) * P, :], ob[:])
                    if all_reduce:
                        ar_dram = dram_pool.tile([DOUT, SQ], F32, tag="arout")
                        nc.gpsimd.collective_compute(
                            "AllReduce",
                            ALU.add,
                            replica_groups=groups,
                            ins=[op_dram.opt()],
                            outs=[ar_dram.opt()],
                        )
                        nc.sync.dma_start(outp[j], ar_dram[:])
                    else:
                        nc.sync.dma_start(outp[j], op_dram[:])

    nc.finalize()
    return nc


def make_perms():
    """perm (q/k): per half, E-block then O-block across the half's 4 heads.
    perm2 (v/wo): per head, [even dims | odd dims].
    Both are local to a core's 512 rows (caller adds the head-group offset)."""
    perm = []
    for half in range(2):
        for par in range(2):  # 0=E, 1=O
            for h in range(4 * half, 4 * half + 4):
                for i in range(32):
                    perm.append(h * DK + 2 * i + par)
    perm2 = []
    for h in range(NH):
        for par in range(2):
            for i in range(32):
                perm2.append(h * DK + 2 * i + par)
    return np.array(perm), np.array(perm2)


def make_tables(token_positions, SEQ):
    pos = np.asarray(token_positions).astype(np.float32)
    inv_freq = (1.0 / (THETA ** (np.arange(0, DK, 2, dtype=np.float32) / DK))).astype(
        np.float32
    )
    freqs = pos[:, None] * inv_freq[None, :]  # [S, 32]
    cosT = np.cos(freqs).T.astype(np.float32)  # [32, S]
    sinT = np.sin(freqs).T.astype(np.float32)
    return (
        np.ascontiguousarray(np.tile(cosT, (4, 1))),
        np.ascontiguousarray(np.tile(sinT, (4, 1))),
    )


def shard_inputs(x, token_positions, wq, wk, wv, wo):
    """Build the 8 per-core input maps."""
    perm, perm2 = make_perms()
    cos4, sin4 = make_tables(token_positions, x.shape[1])
    in_maps = []
    for c in range(N_CORES):
        b, hg = c // 2, c % 2
        rows = hg * DLOC
        gperm = perm + rows
        gperm2 = perm2 + rows
        in_maps.append(
            {
                "xt": np.ascontiguousarray(x[b].T),
                "wqt": np.ascontiguousarray(wq[gperm, :].T),
                "wkt": np.ascontiguousarray(wk[gperm, :].T),
                "wvt": np.ascontiguousarray(wv[gperm2, :].T),
                "wot": np.ascontiguousarray(wo[:, gperm2].T),
                "cos4": cos4,
                "sin4": sin4,
            }
        )
    return in_maps


# ======================================================================
# v2: bf16 datapath, pair-packed PV + col-tiled denominator matmuls,
# 2-bank-wide exps, mask-multiply instead of per-tile affine_select,
# AllGather of attention outputs (split output projection) instead of
# AllReduce of output partials, out-proj software-pipelined by one j.
# ======================================================================

BF16 = mybir.dt.bfloat16


def build_attention_program_v2(SEQ=S, groups=None, reps=1, all_gather=True):
    """Per-core DRAM I/O (bf16 unless noted):
      xt   [D, SEQ]      x[b].T
      wqt  [D, DLOC]     wq rows (perm: per-half E-block/O-block) transposed
      wkt  [D, DLOC]
      wvt  [D, DLOC]     wv rows (perm2: per-head [even|odd]) transposed
      wot  [D, DLOC]     wo[dout half, perm2f cols].T  (full contraction dim)
      cos4 [P, SEQ]      cos table, 4x stacked [32, SEQ]
      sin4 [P, SEQ]
      outp [SEQ//SQ, DLOC, SQ]  per-core half of out^T, j-chunked
    """
    DIN = D
    KC = DIN // P       # 8 contraction chunks for QKV
    SJ = SEQ // SQ      # 4 seq chunks
    STJ = SQ // P       # 4 sk-tiles per chunk
    ST = SEQ // P       # 16 sk-tiles total
    n_dev = (len(groups) * len(groups[0])) if groups else 1

    nc = bacc.Bacc("TRN2", target_bir_lowering=False, debug=False, num_devices=n_dev)
    xt = nc.declare_dram_parameter("xt", [DIN, SEQ], BF16, isOutput=False)
    wqt = nc.declare_dram_parameter("wqt", [DIN, DLOC], BF16, isOutput=False)
    wkt = nc.declare_dram_parameter("wkt", [DIN, DLOC], BF16, isOutput=False)
    wvt = nc.declare_dram_parameter("wvt", [DIN, DLOC], BF16, isOutput=False)
    wot = nc.declare_dram_parameter("wot", [DIN, DLOC], BF16, isOutput=False)
    cos4 = nc.declare_dram_parameter("cos4", [P, SEQ], BF16, isOutput=False)
    sin4 = nc.declare_dram_parameter("sin4", [P, SEQ], BF16, isOutput=False)
    outp = nc.declare_dram_parameter("outp", [SJ, DLOC, SQ], BF16, isOutput=True)

    from contextlib import ExitStack

    with tile.TileContext(nc) as tc, ExitStack() as ctx:
        ctx.enter_context(nc.allow_low_precision(reason="bf16 datapath"))
        consts = ctx.enter_context(tc.tile_pool(name="consts", bufs=1))
        wload = ctx.enter_context(tc.tile_pool(name="wload", bufs=1))
        xload = ctx.enter_context(tc.tile_pool(name="xload", bufs=2))
        qk_pool = ctx.enter_context(tc.tile_pool(name="qk", bufs=1))
        v_pool = ctx.enter_context(tc.tile_pool(name="vp", bufs=1))
        rc_pool = ctx.enter_context(tc.tile_pool(name="rc", bufs=2))
        tmp_pool = ctx.enter_context(tc.tile_pool(name="tmp", bufs=2))
        pt_pool = ctx.enter_context(tc.tile_pool(name="pt", bufs=6))
        den_pool = ctx.enter_context(tc.tile_pool(name="den", bufs=2))
        ot_pool = ctx.enter_context(tc.tile_pool(name="ot", bufs=2))
        og_pool = ctx.enter_context(tc.tile_pool(name="og", bufs=2))
        ob_pool = ctx.enter_context(tc.tile_pool(name="ob", bufs=2))
        dram_pool = ctx.enter_context(tc.tile_pool(name="dram", bufs=2, space="DRAM"))
        psA = ctx.enter_context(tc.tile_pool(name="psA", bufs=1, space="PSUM"))
        psS = ctx.enter_context(tc.tile_pool(name="psS", bufs=2, space="PSUM"))
        psO = ctx.enter_context(tc.tile_pool(name="psO", bufs=2, space="PSUM"))
        psD = ctx.enter_context(tc.tile_pool(name="psD", bufs=1, space="PSUM"))

        for _rep in range(reps):
            ones_f32 = consts.tile([P, 1], F32, tag="one1")
            nc.vector.memset(ones_f32[:], 1.0)
            ones_bf = consts.tile([P, 1], BF16, tag="onebf")
            nc.vector.tensor_copy(ones_bf[:], ones_f32[:])
            zero_f32 = consts.tile([2 * 32, 1], F32, tag="zero1")
            nc.vector.memset(zero_f32[:], 0.0)
            # norm broadcast matrix: out rows 0-63 <- den row 0, 64-127 <- row 32
            z2 = consts.tile([2 * 32, P], F32, tag="z2")
            nc.vector.memset(z2[:], 0.0)
            nc.vector.memset(z2[0:1, 0:DK], 1.0)
            nc.vector.memset(z2[32:33, DK : 2 * DK], 1.0)
            ones2 = consts.tile([2 * 32, P], F32R, tag="ones2")
            nc.vector.tensor_copy(ones2[:], z2[:])
            # causal masks for the 4 diagonal sk-tile offsets r:
            # keep (=1) where col c >= p + 128 r
            mask4 = consts.tile([P, STJ, SQ], BF16, tag="mask4")
            nc.vector.tensor_copy(
                mask4[:], ones_f32[:, 0:1, None].broadcast_to((P, STJ, SQ))
            )
            for r in range(STJ):
                nc.gpsimd.affine_select(
                    out=mask4[:, r, :],
                    in_=mask4[:, r, :],
                    compare_op=ALU.is_ge,
                    fill=0.0,
                    base=-P * r,
                    pattern=[[1, SQ]],
                    channel_multiplier=-1,
                )

            # weights, resident all rep
            wq_sb = wload.tile([P, KC, DLOC], BF16, tag="wq")
            nc.sync.dma_start(wq_sb[:], wqt.rearrange("(k p) c -> p k c", p=P))
            wk_sb = wload.tile([P, KC, DLOC], BF16, tag="wk")
            nc.sync.dma_start(wk_sb[:], wkt.rearrange("(k p) c -> p k c", p=P))
            wv_sb = wload.tile([P, KC, DLOC], BF16, tag="wv")
            nc.sync.dma_start(wv_sb[:], wvt.rearrange("(k p) c -> p k c", p=P))
            wo_sb = wload.tile([P, KC, DLOC], BF16, tag="wo")
            nc.sync.dma_start(wo_sb[:], wot.rearrange("(k p) c -> p k c", p=P))
            cos_sb = consts.tile([P, SEQ], BF16, tag="cos")
            nc.sync.dma_start(cos_sb[:], cos4[:, :])
            sin_sb = consts.tile([P, SEQ], BF16, tag="sin")
            nc.sync.dma_start(sin_sb[:], sin4[:, :])

            # rotated q/k per half in [dout, s] layout, bf16
            qE = [
                qk_pool.tile([P, SEQ], BF16, tag=f"qE{h}", name=f"qE{h}")
                for h in range(2)
            ]
            qO = [
                qk_pool.tile([P, SEQ], BF16, tag=f"qO{h}", name=f"qO{h}")
                for h in range(2)
            ]
            kE = [
                qk_pool.tile([P, SEQ], BF16, tag=f"kE{h}", name=f"kE{h}")
                for h in range(2)
            ]
            kO = [
                qk_pool.tile([P, SEQ], BF16, tag=f"kO{h}", name=f"kO{h}")
                for h in range(2)
            ]
            # v natural [s, dv]: per sk-tile, 8 heads x 64 dims
            v_sb = v_pool.tile([P, ST, NH, DK], BF16, tag="v")

            # out-proj pipelined one j behind; remember (agout, j) to drain
            pending = []

            def do_oproj(agout_t, jj):
                og_sb = og_pool.tile([P, KC, SQ], BF16, tag="og")
                nc.sync.dma_start(
                    og_sb[:], agout_t.rearrange("(k p) s -> p k s", p=P)
                )
                for dc in range(DLOC // P):
                    ps = psA.tile([P, SQ], F32, tag="mm")
                    for ic in range(KC):
                        nc.tensor.matmul(
                            ps[:],
                            lhsT=(wo_sb[:, ic, dc * P : (dc + 1) * P]),
                            rhs=(og_sb[:, ic, :]),
                            start=(ic == 0),
                            stop=(ic == KC - 1),
                        )
                    ob = ob_pool.tile([P, SQ], BF16, tag="ob")
                    nc.vector.tensor_copy(ob[:], ps[:])
                    nc.sync.dma_start(outp[jj, dc * P : (dc + 1) * P, :], ob[:])

            xt_tiles = {}

            def emit_xload(j):
                js = slice(j * SQ, (j + 1) * SQ)
                xt_sb = xload.tile([P, KC, SQ], BF16, tag="xt", name=f"xt{j}")
                nc.sync.dma_start(
                    xt_sb[:], xt[:, js].rearrange("(k p) s -> p k s", p=P)
                )
                xt_tiles[j] = xt_sb

            def emit_qk(j, half):
                js = slice(j * SQ, (j + 1) * SQ)
                xt_sb = xt_tiles[j]
                c0 = half * 256
                for tname, wsb, dE, dO in (
                    ("q", wq_sb, qE[half], qO[half]),
                    ("k", wk_sb, kE[half], kO[half]),
                ):
                    cEO = []
                    for eo in range(2):
                        ps = psA.tile([P, SQ], F32, tag="mm")
                        cc = c0 + eo * P
                        for kk in range(KC):
                            nc.tensor.matmul(
                                ps[:],
                                lhsT=(wsb[:, kk, cc : cc + P]),
                                rhs=(xt_sb[:, kk, :]),
                                start=(kk == 0),
                                stop=(kk == KC - 1),
                            )
                        cx = rc_pool.tile([P, SQ], BF16, tag="cx")
                        nc.vector.tensor_copy(cx[:], ps[:])
                        cEO.append(cx)
                    cE, cO = cEO
                    cos_j = cos_sb[:, js]
                    sin_j = sin_sb[:, js]
                    t1 = tmp_pool.tile([P, SQ], BF16, tag="t1")
                    nc.vector.tensor_tensor(t1[:], cos_j, cE[:], ALU.mult)
                    t2 = tmp_pool.tile([P, SQ], BF16, tag="t2")
                    nc.vector.tensor_tensor(t2[:], sin_j, cO[:], ALU.mult)
                    nc.vector.tensor_tensor(dE[:, js], t1[:], t2[:], ALU.subtract)
                    t3 = tmp_pool.tile([P, SQ], BF16, tag="t1")
                    nc.vector.tensor_tensor(t3[:], sin_j, cE[:], ALU.mult)
                    t4 = tmp_pool.tile([P, SQ], BF16, tag="t2")
                    nc.vector.tensor_tensor(t4[:], cos_j, cO[:], ALU.mult)
                    nc.vector.tensor_tensor(dO[:, js], t3[:], t4[:], ALU.add)

            def emit_v(j):
                xt_sb = xt_tiles[j]
                for st in range(STJ):
                    ps = psA.tile([P, DLOC], F32, tag="mm")
                    for kk in range(KC):
                        nc.tensor.matmul(
                            ps[:],
                            lhsT=(xt_sb[:, kk, st * P : (st + 1) * P]),
                            rhs=(wv_sb[:, kk, :]),
                            start=(kk == 0),
                            stop=(kk == KC - 1),
                        )
                    nc.vector.tensor_copy(
                        v_sb[:, j * STJ + st, :, :],
                        ps.rearrange("p (h d) -> p h d", h=NH),
                    )

            def emit_attn_half(j, half, ot_j):
                js = slice(j * SQ, (j + 1) * SQ)
                ntile = (j + 1) * STJ
                if True:
                    opvs = [
                        psO.tile([P, SQ], F32, tag="pv", name=f"pv{half}{i}")
                        for i in range(2)
                    ]
                    den_ps = psD.tile([P, SQ], F32, tag="dn", name=f"dn{half}")

                    def emit_scores(t):
                        ts_ = slice(t * P, (t + 1) * P)
                        pts = []
                        for i in range(2):  # head pair
                            ssc = psS.tile([P, 2, SQ], F32, tag="sc")
                            for hh in range(2):
                                bp = (i * 2 + hh) * 32
                                nc.tensor.matmul(
                                    ssc[:, hh, :],
                                    lhsT=(kE[half][bp : bp + 32, ts_]),
                                    rhs=(qE[half][bp : bp + 32, js]),
                                    start=True,
                                    stop=False,
                                    tile_position=(bp, 0),
                                )
                                nc.tensor.matmul(
                                    ssc[:, hh, :],
                                    lhsT=(kO[half][bp : bp + 32, ts_]),
                                    rhs=(qO[half][bp : bp + 32, js]),
                                    start=False,
                                    stop=True,
                                    tile_position=(bp, 0),
                                )
                            pt = pt_pool.tile([P, 2, SQ], BF16, tag="pt")
                            nc.scalar.activation(pt[:], ssc[:], AF.Exp, scale=0.125)
                            if t >= ntile - STJ:
                                r = t - STJ * j
                                ptm = pt_pool.tile([P, 2, SQ], BF16, tag="pt")
                                nc.vector.tensor_tensor(
                                    ptm[:],
                                    pt[:],
                                    mask4[:, r : r + 1, :].broadcast_to((P, 2, SQ)),
                                    ALU.mult,
                                )
                                pt = ptm
                            pts.append(pt)
                        return pts

                    def emit_pv(t, pts):
                        for i in range(2):
                            for hh in range(2):
                                hloc = i * 2 + hh
                                nc.tensor.matmul(
                                    opvs[i][hh * DK : (hh + 1) * DK, :],
                                    lhsT=(v_sb[:, t, half * 4 + hloc, :]),
                                    rhs=(pts[i][:, hh, :]),
                                    start=(t == 0),
                                    stop=(t == ntile - 1),
                                    tile_position=(0, hh * DK),
                                )
                                nc.tensor.matmul(
                                    den_ps[hloc * 32 : hloc * 32 + 1, :],
                                    lhsT=(ones_bf[:]),
                                    rhs=(pts[i][:, hh, :]),
                                    start=(t == 0),
                                    stop=(t == ntile - 1),
                                    tile_position=(0, hloc * 32),
                                )

                    # software pipeline: scores(t+1) issue before PV(t) so the
                    # PE never queue-blocks on exp(t)
                    prev = emit_scores(0)
                    for t in range(1, ntile):
                        cur = emit_scores(t)
                        emit_pv(t - 1, prev)
                        prev = cur
                    emit_pv(ntile - 1, prev)
                    # normalize: ot rows of pair i <- opv / den
                    for i in range(2):
                        den_sb = den_pool.tile([2 * 32, SQ], F32R, tag="dsb")
                        nc.vector.tensor_copy(
                            den_sb[:], zero_f32[:, 0:1].broadcast_to((2 * 32, SQ))
                        )
                        for hh in range(2):
                            hloc = i * 2 + hh
                            nc.vector.reciprocal(
                                den_sb[hh * 32 : hh * 32 + 1, :],
                                den_ps[hloc * 32 : hloc * 32 + 1, :],
                            )
                        psb = psS.tile([P, SQ], F32, tag="sc")
                        nc.tensor.matmul(
                            psb[:], lhsT=(ones2[:]), rhs=(den_sb[:]),
                            start=True, stop=True,
                        )
                        denb = den_pool.tile([P, SQ], F32, tag="denb")
                        nc.vector.tensor_copy(denb[:], psb[:])
                        nc.vector.tensor_tensor(
                            ot_j[:, half * 2 + i, :], opvs[i][:], denb[:], ALU.mult
                        )

            def emit_exchange(j, ot_j):
                if all_gather:
                    agin = dram_pool.tile([DLOC, SQ], BF16, tag="agin")
                    nc.sync.dma_start(
                        agin.rearrange("(c p) s -> p c s", p=P), ot_j[:]
                    )
                    agout = dram_pool.tile([2 * DLOC, SQ], BF16, tag="agout")
                    nc.gpsimd.collective_compute(
                        "AllGather",
                        ALU.bypass,
                        replica_groups=groups,
                        ins=[agin.opt()],
                        outs=[agout.opt()],
                    )
                    pending.append((agout, j))
                    if len(pending) > 1:
                        do_oproj(*pending.pop(0))
                else:
                    # single-core testing: duplicate own half
                    agout = dram_pool.tile([2 * DLOC, SQ], BF16, tag="agout")
                    nc.sync.dma_start(
                        agout[0:DLOC].rearrange("(c p) s -> p c s", p=P), ot_j[:]
                    )
                    nc.sync.dma_start(
                        agout[DLOC : 2 * DLOC].rearrange("(c p) s -> p c s", p=P),
                        ot_j[:],
                    )
                    pending.append((agout, j))
                    if len(pending) > 1:
                        do_oproj(*pending.pop(0))

            # ---- pipelined schedule: QKV(j+1) interleaves attention(j) ----
            emit_xload(0)
            emit_qk(0, 0)
            emit_qk(0, 1)
            emit_v(0)
            for j in range(SJ):
                if j + 1 < SJ:
                    emit_xload(j + 1)
                ot_j = ot_pool.tile([P, 4, SQ], BF16, tag="otj", name=f"otj{j}")
                emit_attn_half(j, 0, ot_j)
                if j + 1 < SJ:
                    emit_qk(j + 1, 0)
                emit_attn_half(j, 1, ot_j)
                if j + 1 < SJ:
                    emit_qk(j + 1, 1)
                    emit_v(j + 1)
                emit_exchange(j, ot_j)
                xt_tiles.pop(j, None)

            while pending:
                do_oproj(*pending.pop(0))

    nc.finalize()
    return nc


def make_perm2f():
    return np.array(
        [gh * DK + 2 * i + par for gh in range(H) for par in range(2) for i in range(32)]
    )


def shard_inputs_v2(x, token_positions, wq, wk, wv, wo):
    import ml_dtypes

    bf = ml_dtypes.bfloat16
    perm, perm2 = make_perms()
    perm2f = make_perm2f()
    cos4, sin4 = make_tables(token_positions, x.shape[1])
    in_maps = []
    for c in range(N_CORES):
        b, hg = c // 2, c % 2
        rows = hg * DLOC
        gperm = perm + rows
        gperm2 = perm2 + rows
        douts = np.arange(hg * DLOC, (hg + 1) * DLOC)
        in_maps.append(
            {
                "xt": np.ascontiguousarray(x[b].T).astype(bf),
                "wqt": np.ascontiguousarray(wq[gperm, :].T).astype(bf),
                "wkt": np.ascontiguousarray(wk[gperm, :].T).astype(bf),
                "wvt": np.ascontiguousarray(wv[gperm2, :].T).astype(bf),
                "wot": np.ascontiguousarray(wo[np.ix_(douts, perm2f)].T).astype(bf),
                "cos4": cos4.astype(bf),
                "sin4": sin4.astype(bf),
            }
        )
    return in_maps


def unshard_output_v2(res_list):
    out = np.empty((B, S, D), dtype=np.float32)
    for b in range(B):
        outT = np.empty((D, S), dtype=np.float32)
        for hg in range(2):
            chunks = res_list[2 * b + hg]["outp"]  # [SJ, DLOC, SQ] bf16
            for j in range(S // SQ):
                outT[hg * DLOC : (hg + 1) * DLOC, j * SQ : (j + 1) * SQ] = np.asarray(
                    chunks[j], dtype=np.float32
                )
        out[b] = outT.T
    return out


_NC_CACHE = {}


def kernel(x, token_positions, wq, wk, wv, wo, trace=False):
    x = np.asarray(x, dtype=np.float32)
    wq = np.asarray(wq, dtype=np.float32)
    wk = np.asarray(wk, dtype=np.float32)
    wv = np.asarray(wv, dtype=np.float32)
    wo = np.asarray(wo, dtype=np.float32)

    key = "v2"
    if key not in _NC_CACHE:
        _NC_CACHE[key] = build_attention_program_v2(
            SEQ=S,
            groups=[[0, 1], [2, 3], [4, 5], [6, 7]],
        )
    nc = _NC_CACHE[key]

    in_maps = shard_inputs_v2(x, token_positions, wq, wk, wv, wo)
    res = run_bass_kernel_spmd(nc, in_maps, list(range(N_CORES)), trace=trace)
    out = unshard_output_v2(res.results)
    if trace:
        return out, res
    return out

